# revision 1
# baseline (speedup 1.0000x reference)
"""BWGNN (Bernstein-wavelet GNN) Trainium2 kernel, 8-core SPMD.

Sharding: nodes split 8 ways (graph/data parallel); edges partitioned by dst
shard; tiny weights replicated.  Per round of Laplacian message passing the
node-state table (dinv * f) is AllGathered, then per-edge src rows are
fetched with dma_gather (int16 indices -> the global table is addressed in
<=25344-row chunks; <=1024 indices per instruction, the HW SWDGE ring cap).

Segment-sum by dst runs on the TensorEngine: edges are sorted by (src-chunk,
dst-window-of-128) and packed contiguously (a 128-edge tile may straddle two
windows); the VectorEngine builds one-hot indicators [128e, 128dst] from
host-precomputed window-relative dst values (iota vs dstw, step-0 broadcast
APs; out-of-window edges self-mask to zero rows), and matmuls accumulate
each window in PSUM, added into an SBUF-resident agg.
This avoids dma_scatter_add entirely (its CCE read-modify-write races on
duplicate indices, and its descriptor generation was half the POOL time).

MLP in/out runs feature-major with stationary-weight matmuls; node-major
states are produced by PE transposes; the three Bernstein filters are fused
into the transposes with scaled identity matrices.  Outputs are written
feature-major [64, shard]; the host transposes and concatenates.
Measured: 4.23 ms exec (neuron-profile), rel err 6.5e-7 vs the jax reference.
"""

import sys
from contextlib import ExitStack

import numpy as np

try:
    import concourse  # noqa: F401
except ImportError:  # pragma: no cover
    sys.path.insert(0, "/opt/trn_rl_repo")

import concourse.bacc as bacc
import concourse.bass as bass
import concourse.mybir as mybir
import concourse.tile as tile
from concourse.bass_utils import run_bass_kernel_spmd
from concourse.library_config import mlp
from concourse.masks import make_identity

P = 128
F32 = mybir.dt.float32
I16 = mybir.dt.int16


class Cfg:
    def __init__(self, n_nodes, n_edges, in_feats, h_feats, n_cores,
                 max_span_tiles=8, nbuf=4, mm_chunk=512):
        assert n_nodes % n_cores == 0
        self.n_nodes, self.n_edges = n_nodes, n_edges
        self.in_feats, self.h = in_feats, h_feats
        self.nc = n_cores
        self.shard = n_nodes // n_cores
        self.sp = ((self.shard + P - 1) // P) * P      # padded shard
        self.t = self.sp // P                          # node tiles
        self.tp = self.sp + P                          # table rows/core
        self.tbl_rows = self.tp * n_cores
        self.n_chunks = max(1, (self.tbl_rows + 25343) // 25344)
        self.chunk_shards = -(-n_cores // self.n_chunks)  # shards per chunk
        self.chunk = self.chunk_shards * self.tp
        assert self.chunk <= 32640, self.chunk
        self.n_chunks = -(-n_cores // self.chunk_shards)
        self.max_span_tiles = max_span_tiles           # per-instr tile cap
        self.nbuf = nbuf
        self.mm_chunk = mm_chunk


# ---------------------------------------------------------------- host prep

def _per_core_groups(cfg, srcrow, dstloc):
    """-> dict (chunk, window) -> (src_local_i16[], dst_in_window_f32[])."""
    q = srcrow // cfg.chunk
    out = {}
    for c in range(cfg.n_chunks):
        m = q == c
        sc = (srcrow[m] - c * cfg.chunk).astype(np.int64)
        dc = dstloc[m].astype(np.int64)
        o = np.argsort(dc, kind="stable")
        sc, dc = sc[o], dc[o]
        w = dc // P
        for ww in np.unique(w):
            sel = w == ww
            out[(c, int(ww))] = (sc[sel].astype(np.int16),
                                 (dc[sel] % P).astype(np.float32))
    return out


def _wrap16(x):
    """flat int16 stream -> [128, n/16]: storage[p, col] = x[col*16 + p%16]."""
    assert len(x) % 16 == 0
    return np.tile(x.reshape(-1, 16).T, (8, 1)).copy()


def preprocess(cfg, in_feat, src, dst, W1, b1, W2, b2, W3, b3, W4, b4):
    n = cfg.n_nodes
    deg = np.bincount(dst, minlength=n).astype(np.float32)
    dinv = np.clip(deg, 1.0, None) ** -0.5

    shard_of = dst // cfg.shard
    groups = []
    for c in range(cfg.nc):
        m = shard_of == c
        src_c, dst_c = src[m].astype(np.int64), dst[m].astype(np.int64)
        srcrow = (src_c // cfg.shard) * cfg.tp + (src_c % cfg.shard)
        dstloc = dst_c - c * cfg.shard
        groups.append(_per_core_groups(cfg, srcrow, dstloc))

    # canonical (chunk, window) sizes = max over cores; groups packed
    # contiguously (tiles may straddle two windows; a straddle tile is
    # matmul'd once per window with self-masking indicators).
    keys = sorted(set().union(*[set(g.keys()) for g in groups]))
    sizes = {k: max(len(g.get(k, ((), ()))[0]) for g in groups) for k in keys}
    # lay out a flat stream per chunk: (chunk, window) -> [start, end)
    layout = {}
    chunk_len = {}
    for c in range(cfg.n_chunks):
        pos = 0
        tile_wins = {}
        for (cc, ww) in keys:
            if cc != c:
                continue
            t0 = pos // P
            if len(tile_wins.get(t0, ())) >= 2 and pos % P:
                pos = (t0 + 1) * P          # avoid 3-window tiles
            layout[(c, ww)] = (pos, pos + sizes[(c, ww)])
            for t in range(pos // P, (pos + sizes[(c, ww)] - 1) // P + 1):
                tile_wins.setdefault(t, []).append(ww)
            pos += sizes[(c, ww)]
        chunk_len[c] = -(-pos // P) * P     # round chunk stream to tiles
        layout[("tw", c)] = tile_wins

    # bundles of <= max_span_tiles tiles per chunk; per-bundle matmul events
    plan = []  # (chunk, btiles, goff_tiles, [(w, [(tile_in_bundle, sel)])])
    goff = 0
    for c in range(cfg.n_chunks):
        tiles = chunk_len[c] // P
        tile_wins = layout[("tw", c)]
        b0 = 0
        while b0 < tiles:
            bt = min(cfg.max_span_tiles, tiles - b0)
            events = {}
            for t in range(b0, b0 + bt):
                for si, ww in enumerate(tile_wins.get(t, [])):
                    events.setdefault(ww, []).append((t - b0, si + 1))
            plan.append((c, bt, goff, sorted(events.items())))
            goff += bt
            b0 += bt
    total_tiles = goff

    total_tiles = sum(b[1] for b in plan)
    in_maps = []
    zero_local = [s * cfg.tp + cfg.sp for s in range(cfg.nc)]

    def chunk_zero(c):
        for z in zero_local:
            if c * cfg.chunk <= z < (c + 1) * cfg.chunk:
                return z - c * cfg.chunk
        raise AssertionError("no zero row in chunk")

    for core in range(cfg.nc):
        gz = np.zeros(total_tiles * P, np.int16)
        dw1 = np.full(total_tiles * P, 999.0, np.float32)
        dw2 = np.full(total_tiles * P, 999.0, np.float32)
        cbase = {}
        acc_t = 0
        for c in range(cfg.n_chunks):
            cbase[c] = acc_t * P
            acc_t += chunk_len[c] // P
        for (c, ww) in keys:
            start, end = layout[(c, ww)]
            s_arr, d_arr = groups[core].get(
                (c, ww), (np.zeros(0, np.int16), np.zeros(0, np.float32)))
            o = cbase[c] + start
            gz[o:o + (end - start)] = chunk_zero(c)
            gz[o:o + len(s_arr)] = s_arr
            # dstw relative to each slot's tile's first/second window
            tile_wins = layout[("tw", c)]
            for i in range(end - start):
                pass  # vectorized below
            sl = np.arange(start, end)
            tl = sl // P
            first = np.array([tile_wins[t][0] for t in tl])
            dl_full = np.full(end - start, 999.0 + 128000.0, np.float64)
            dl_full[:len(d_arr)] = d_arr + 128.0 * ww
            v1 = dl_full - 128.0 * first
            v1[(v1 < 0) | (v1 >= P) | (dl_full > 90000)] = 999.0
            second = np.array([tile_wins[t][1] if len(tile_wins[t]) > 1
                               else -999 for t in tl])
            v2 = dl_full - 128.0 * second
            v2[(v2 < 0) | (v2 >= P) | (dl_full > 90000)] = 999.0
            dw1[o:o + (end - start)] = v1
            dw2[o:o + (end - start)] = v2
        lo, hi = core * cfg.shard, (core + 1) * cfg.shard
        xT = np.zeros((cfg.in_feats, cfg.sp), np.float32)
        xT[:, :cfg.shard] = in_feat[lo:hi].T
        full = np.ones(cfg.sp, np.float32)
        full[:cfg.shard] = dinv[lo:hi]
        dpm = np.ascontiguousarray(full.reshape(cfg.t, P).T)
        # dstw layout [128, total_tiles]: [p, t] = value of edge slot t*128+p
        dwt1 = np.ascontiguousarray(dw1.reshape(total_tiles, P).T)
        dwt2 = np.ascontiguousarray(dw2.reshape(total_tiles, P).T)
        in_maps.append({
            "xT": xT, "dinv_pm": dpm,
            "gidx": _wrap16(gz), "dstw1": dwt1, "dstw2": dwt2,
            "W1": np.asarray(W1, np.float32), "W2": np.asarray(W2, np.float32),
            "W3": np.asarray(W3, np.float32), "W4": np.asarray(W4, np.float32),
            "b1": np.asarray(b1, np.float32).reshape(-1, 1),
            "b2": np.asarray(b2, np.float32).reshape(-1, 1),
            "b3": np.asarray(b3, np.float32).reshape(-1, 1),
            "b4": np.asarray(b4, np.float32).reshape(-1, 1),
        })
    return in_maps, plan, total_tiles


# ---------------------------------------------------------------- builder

def build_nc(cfg, plan, total_tiles):
    H = cfg.h
    idx_cols = total_tiles * 8
    nc = bacc.Bacc("TRN2", target_bir_lowering=False, debug=False,
                   num_devices=cfg.nc)
    xT_d = nc.dram_tensor("xT", [cfg.in_feats, cfg.sp], F32, kind="ExternalInput")
    dinv_d = nc.dram_tensor("dinv_pm", [P, cfg.t], F32, kind="ExternalInput")
    gidx_d = nc.dram_tensor("gidx", [P, idx_cols], I16, kind="ExternalInput")
    dstw1_d = nc.dram_tensor("dstw1", [P, total_tiles], F32, kind="ExternalInput")
    dstw2_d = nc.dram_tensor("dstw2", [P, total_tiles], F32, kind="ExternalInput")
    W_d = {w: nc.dram_tensor(w, [cfg.in_feats if w in ("W1", "W4") else H, H],
                             F32, kind="ExternalInput")
           for w in ("W1", "W2", "W3", "W4")}
    b_d = {b: nc.dram_tensor(b, [H, 1], F32, kind="ExternalInput")
           for b in ("b1", "b2", "b3", "b4")}
    outl_d = nc.dram_tensor("out_l", [H, cfg.sp], F32, kind="ExternalOutput")
    outh_d = nc.dram_tensor("out_h", [H, cfg.sp], F32, kind="ExternalOutput")

    relu = mybir.ActivationFunctionType.Relu
    cp = mybir.ActivationFunctionType.Copy

    with tile.TileContext(nc) as tc, ExitStack() as ctx:
        pers = ctx.enter_context(tc.tile_pool(name="pers", bufs=1))
        dram = ctx.enter_context(tc.tile_pool(name="dram", bufs=1, space="DRAM"))
        io = ctx.enter_context(tc.tile_pool(name="io", bufs=2))
        one = ctx.enter_context(tc.tile_pool(name="one", bufs=1))
        idxp = ctx.enter_context(tc.tile_pool(name="idxp", bufs=6))
        gbp = ctx.enter_context(tc.tile_pool(name="gbp", bufs=6))
        gbi = ctx.enter_context(tc.tile_pool(name="gbi", bufs=2))
        psum = ctx.enter_context(tc.tile_pool(name="psum", bufs=2, space="PSUM"))
        psum1 = ctx.enter_context(tc.tile_pool(name="psum1", bufs=2, space="PSUM"))
        psum2 = ctx.enter_context(tc.tile_pool(name="psum2", bufs=2, space="PSUM"))

        nc.gpsimd.load_library(mlp)

        f0 = pers.tile([P, cfg.t, 64], F32, tag="f0")
        f1 = pers.tile([P, cfg.t, 64], F32, tag="f1")
        f2 = pers.tile([P, cfg.t, 64], F32, tag="f2")
        tbl = pers.tile([P, cfg.t + 1, 64], F32, tag="tbl")
        dinv_s = pers.tile([P, cfg.t], F32, tag="dinv")
        Ws = {w: pers.tile([cfg.in_feats if w in ("W1", "W4") else H, H],
                           F32, tag=w, name=w + "_s")
              for w in ("W1", "W2", "W3", "W4")}
        bs = {b: pers.tile([H, 1], F32, tag=b, name=b + "_s")
              for b in ("b1", "b2", "b3", "b4")}
        ident = pers.tile([P, P], F32, tag="ident")
        sid3 = pers.tile([P, P], F32, tag="sid3")
        sid075 = pers.tile([P, P], F32, tag="sid075")
        sidm15 = pers.tile([P, P], F32, tag="sidm15")

        tb_ins = [dram.tile([cfg.tp, 64], F32, name=f"tb_in{r}")
                  for r in range(2)]
        tb_fulls = [dram.tile([cfg.tp * cfg.nc, 64], F32, addr_space="Shared",
                              name=f"tb_full{r}") for r in range(2)]
        agg = pers.tile([P, cfg.t, 64], F32, tag="agg")
        iota_f = pers.tile([P, P], F32, tag="iota_f")

        for w in Ws:
            nc.sync.dma_start(Ws[w][:], W_d[w][:])
        for b in bs:
            nc.sync.dma_start(bs[b][:], b_d[b][:])
        nc.sync.dma_start(dinv_s[:], dinv_d[:])
        make_identity(nc, ident[:])
        nc.vector.tensor_scalar_mul(sid3[:], ident[:], 3.0)
        nc.vector.tensor_scalar_mul(sid075[:], ident[:], 0.75)
        nc.vector.tensor_scalar_mul(sidm15[:], ident[:], -1.5)
        nc.gpsimd.memset(tbl[:, cfg.t, :], 0.0)
        ioti = pers.tile([P, P], mybir.dt.int32, tag="ioti")
        nc.gpsimd.iota(ioti[:], pattern=[[1, P]], base=0, channel_multiplier=0)
        nc.vector.tensor_copy(iota_f[:], ioti[:])

        # ---- phase 1: MLP -> f0 node-major
        CH = cfg.mm_chunk
        for j0 in range(0, cfg.sp, CH):
            w = min(CH, cfg.sp - j0)
            xc = io.tile([cfg.in_feats, CH], F32, tag="xc")
            nc.sync.dma_start(xc[:, :w], xT_d[:, j0:j0 + w])
            ps1 = psum.tile([H, CH], F32, tag="A")
            nc.tensor.matmul(ps1[:, :w], Ws["W1"][:], xc[:, :w],
                             start=True, stop=True)
            h1c = io.tile([H, CH], F32, tag="h1c")
            nc.scalar.activation(h1c[:, :w], ps1[:, :w], relu, bias=bs["b1"][:])
            ps2 = psum.tile([H, CH], F32, tag="B")
            nc.tensor.matmul(ps2[:, :w], Ws["W2"][:], h1c[:, :w],
                             start=True, stop=True)
            h2c = io.tile([H, CH], F32, tag="h2c")
            nc.scalar.activation(h2c[:, :w], ps2[:, :w], relu, bias=bs["b2"][:])
            for i in range(w // P):
                t = (j0 + i * P) // P
                ps3 = psum1.tile([P, 64], F32, tag="C")
                nc.tensor.transpose(ps3[:], h2c[:, i * P:(i + 1) * P],
                                    ident[:H, :H])
                nc.scalar.activation(f0[:, t, :], ps3[:], cp)

        # ---- message passing rounds
        for rnd, (fprev, fnext) in enumerate([(f0, f1), (f1, f2)]):
            tb_in, tb_full = tb_ins[rnd], tb_fulls[rnd]
            nc.vector.tensor_tensor(
                tbl[:, :cfg.t, :], fprev[:],
                dinv_s[:, :, None].to_broadcast([P, cfg.t, 64]),
                mybir.AluOpType.mult)
            nc.sync.dma_start(
                tb_in[:].rearrange("(t p) f -> p t f", p=P), tbl[:])
            nc.gpsimd.collective_compute(
                "AllGather", mybir.AluOpType.bypass,
                replica_groups=[list(range(cfg.nc))],
                ins=[tb_in[:]], outs=[tb_full[:]])
            nc.gpsimd.memset(agg[:], 0.0)
            for (c, btiles, goff, events) in plan:
                gi = idxp.tile([P, cfg.max_span_tiles * 8], I16, tag="gi")
                dv1 = idxp.tile([P, cfg.max_span_tiles], F32, tag="dv1")
                dv2 = idxp.tile([P, cfg.max_span_tiles], F32, tag="dv2")
                nc.sync.dma_start(gi[:, :btiles * 8],
                                  gidx_d[:, goff * 8:(goff + btiles) * 8])
                nc.sync.dma_start(dv1[:, :btiles],
                                  dstw1_d[:, goff:goff + btiles])
                nc.sync.dma_start(dv2[:, :btiles],
                                  dstw2_d[:, goff:goff + btiles])
                gb = gbp.tile([P, cfg.max_span_tiles, 64], F32, tag="gb")
                ni = btiles * P
                nc.gpsimd.dma_gather(
                    gb[:, :btiles, :],
                    tb_full[c * cfg.chunk:(c + 1) * cfg.chunk, :],
                    gi[:, :btiles * 8], ni, ni, 64)
                ind1 = gbi.tile([P, cfg.max_span_tiles, P], F32, tag="ind1")
                ind2 = gbi.tile([P, cfg.max_span_tiles, P], F32, tag="ind2")
                nc.vector.tensor_tensor(
                    ind1[:, :btiles, :],
                    iota_f[:, None, :].to_broadcast([P, btiles, P]),
                    dv1[:, :btiles, None].to_broadcast([P, btiles, P]),
                    mybir.AluOpType.is_equal)
                need2 = any(s == 2 for _, tl in events for _, s in tl)
                if need2:
                    nc.vector.tensor_tensor(
                        ind2[:, :btiles, :],
                        iota_f[:, None, :].to_broadcast([P, btiles, P]),
                        dv2[:, :btiles, None].to_broadcast([P, btiles, P]),
                        mybir.AluOpType.is_equal)
                for (ww, tl) in events:
                    pw = psum2.tile([P, 64], F32, tag="D")
                    for i, (t, sel) in enumerate(tl):
                        ind = ind1 if sel == 1 else ind2
                        nc.tensor.matmul(pw[:], ind[:, t, :], gb[:, t, :],
                                         start=(i == 0), stop=(i == len(tl) - 1))
                    nc.vector.tensor_tensor(agg[:, ww, :], agg[:, ww, :],
                                            pw[:], mybir.AluOpType.add)
            # sliced per 4-window group so filter chunks can start while
            # late gather bundles are still in flight
            for c4 in range(0, cfg.t, 4):
                hi4 = min(c4 + 4, cfg.t)
                w4 = hi4 - c4
                nc.vector.tensor_tensor(
                    fnext[:, c4:hi4, :], agg[:, c4:hi4, :],
                    dinv_s[:, c4:hi4, None].to_broadcast([P, w4, 64]),
                    mybir.AluOpType.mult)
                nc.vector.tensor_tensor(fnext[:, c4:hi4, :],
                                        fprev[:, c4:hi4, :],
                                        fnext[:, c4:hi4, :],
                                        mybir.AluOpType.subtract)

        # ---- filters + output MLPs
        nc.vector.tensor_tensor(f0[:], f0[:], f1[:], mybir.AluOpType.subtract)
        for j0 in range(0, cfg.sp, CH):
            w = min(CH, cfg.sp - j0)
            zl = psum.tile([H, CH], F32, tag="A")
            z1 = psum.tile([H, CH], F32, tag="B")
            z2 = psum1.tile([H, CH], F32, tag="C")
            for i in range(w // P):
                t = (j0 + i * P) // P
                cs = slice(i * P, (i + 1) * P)
                nc.tensor.matmul(zl[:, cs], f0[:, t, :], sid3[:],
                                 start=True, stop=False)
                nc.tensor.matmul(zl[:, cs], f2[:, t, :], sid075[:],
                                 start=False, stop=True)
                nc.tensor.matmul(z1[:, cs], f1[:, t, :], sid3[:],
                                 start=True, stop=False)
                nc.tensor.matmul(z1[:, cs], f2[:, t, :], sidm15[:],
                                 start=False, stop=True)
                nc.tensor.matmul(z2[:, cs], f2[:, t, :], sid075[:],
                                 start=True, stop=True)
            zlc = io.tile([H, CH], F32, tag="zlc")
            zhc = io.tile([P, CH], F32, tag="zhc")
            nc.scalar.activation(zlc[:, :w], zl[:, :w], cp)
            nc.scalar.activation(zhc[:H, :w], z1[:, :w], cp)
            nc.scalar.activation(zhc[H:, :w], z2[:, :w], cp)
            pl = psum1.tile([H, CH], F32, tag="C")
            ph = psum.tile([H, CH], F32, tag="A")
            nc.tensor.matmul(pl[:, :w], Ws["W3"][:], zlc[:, :w],
                             start=True, stop=True)
            nc.tensor.matmul(ph[:, :w], Ws["W4"][:], zhc[:, :w],
                             start=True, stop=True)
            ol = io.tile([H, CH], F32, tag="ol")
            oh = io.tile([H, CH], F32, tag="oh")
            nc.scalar.activation(ol[:, :w], pl[:, :w], relu, bias=bs["b3"][:])
            nc.scalar.activation(oh[:, :w], ph[:, :w], relu, bias=bs["b4"][:])
            nc.sync.dma_start(outl_d[:, j0:j0 + w], ol[:, :w])
            nc.sync.dma_start(outh_d[:, j0:j0 + w], oh[:, :w])

    nc.compile()
    return nc


# ---------------------------------------------------------------- driver

_CACHE = {}


def run(cfg, inputs, run_fn=None, **spmd_kwargs):
    in_maps, plan, total_tiles = preprocess(cfg, **inputs)
    key = (cfg.n_nodes, cfg.n_edges, total_tiles, repr(plan))
    if key not in _CACHE:
        _CACHE[key] = build_nc(cfg, plan, total_tiles)
    nc = _CACHE[key]
    if run_fn is not None:
        results = run_fn(nc, in_maps)
        res = None
    else:
        res = run_bass_kernel_spmd(nc, in_maps, core_ids=list(range(cfg.nc)), **spmd_kwargs)
        results = res.results
    h_l = np.zeros((cfg.n_nodes, cfg.h), np.float32)
    h_h = np.zeros((cfg.n_nodes, cfg.h), np.float32)
    for c in range(cfg.nc):
        lo = c * cfg.shard
        h_l[lo:lo + cfg.shard] = results[c]["out_l"].T[:cfg.shard]
        h_h[lo:lo + cfg.shard] = results[c]["out_h"].T[:cfg.shard]
    return h_l, h_h, res


def kernel(in_feat, src, dst, W1, b1, W2, b2, W3, b3, W4, b4):
    cfg = Cfg(100000, 1600000, 128, 64, 8)
    h_l, h_h, _ = run(cfg, dict(
        in_feat=np.asarray(in_feat, np.float32),
        src=np.asarray(src, np.int64), dst=np.asarray(dst, np.int64),
        W1=np.asarray(W1, np.float32), b1=np.asarray(b1, np.float32),
        W2=np.asarray(W2, np.float32), b2=np.asarray(b2, np.float32),
        W3=np.asarray(W3, np.float32), b3=np.asarray(b3, np.float32),
        W4=np.asarray(W4, np.float32), b4=np.asarray(b4, np.float32)))
    return h_l, h_h



# revision 13
# speedup vs baseline: 1.2804x; 1.2804x over previous
"""BWGNN (Bernstein-wavelet GNN) Trainium2 kernel, 8-core SPMD.

Sharding: nodes split 8 ways (graph/data parallel); edges partitioned by dst
shard; tiny weights replicated.  Per round of Laplacian message passing the
node-state table (dinv * f, bf16, rows padded to 256B) is AllGathered, then
per-edge src rows are fetched with dma_gather (int16 indices; the global
table is addressed in <=25344-row chunks; 2048 indices per instruction).

Gather descriptor generation on the Pool engine (~9.4ns/idx ucode) is the
hard bottleneck, so the edge-slot layout minimizes index count: per-core
streams are padded only per chunk (max over cores, ~0.7%) instead of per
(chunk, window) group.  Windows float: each canonical tile carries up to 4
candidate dst windows (union over cores); per-tile window-relative dst
values (4 host-precomputed bf16 streams) turn into one-hot indicators on
the VectorEngine (iota is_eq, self-masking out-of-window edges), and the
TensorEngine accumulates each window event in PSUM, first-touch-copied or
added into an SBUF fp32 agg.  All matmul operands are bf16 (PSUM fp32).

MLP in/out runs feature-major with stationary-weight matmuls; node-major
states via PE transposes; the three Bernstein filters fold into scaled
identity matrices.  Outputs are written feature-major fp32 [64, shard];
the host transposes and concatenates.
"""

import sys
from contextlib import ExitStack

import numpy as np
import ml_dtypes

try:
    import concourse  # noqa: F401
except ImportError:  # pragma: no cover
    sys.path.insert(0, "/opt/trn_rl_repo")

import concourse.bacc as bacc
import concourse.bass as bass
import concourse.mybir as mybir
import concourse.tile as tile
from concourse.bass_utils import run_bass_kernel_spmd
from concourse.library_config import mlp
from concourse.masks import make_identity

P = 128
F32 = mybir.dt.float32
BF16 = mybir.dt.bfloat16
I16 = mybir.dt.int16
BF = ml_dtypes.bfloat16
NSTREAM = 4


class Cfg:
    def __init__(self, n_nodes, n_edges, in_feats, h_feats, n_cores,
                 max_span_tiles=8, mm_chunk=512):
        assert n_nodes % n_cores == 0
        self.n_nodes, self.n_edges = n_nodes, n_edges
        self.in_feats, self.h = in_feats, h_feats
        self.nc = n_cores
        self.shard = n_nodes // n_cores
        self.sp = ((self.shard + P - 1) // P) * P      # padded shard
        self.t = self.sp // P                          # node tiles
        self.tp = self.sp + P                          # table rows/core
        self.tbl_rows = self.tp * n_cores
        self.chunk_shards = 2
        self.chunk = self.chunk_shards * self.tp
        assert self.chunk <= 32640, self.chunk
        self.n_chunks = -(-n_cores // self.chunk_shards)
        self.max_span_tiles = max_span_tiles
        self.mm_chunk = mm_chunk


# ---------------------------------------------------------------- host prep

def _wrap16(x):
    """flat int16 stream -> [128, n/16]: storage[p, col] = x[col*16 + p%16]."""
    assert len(x) % 16 == 0
    return np.tile(x.reshape(-1, 16).T, (8, 1)).copy()


def preprocess(cfg, in_feat, src, dst, W1, b1, W2, b2, W3, b3, W4, b4):
    n = cfg.n_nodes
    deg = np.bincount(dst, minlength=n).astype(np.float32)
    dinv = np.clip(deg, 1.0, None) ** -0.5

    shard_of = dst // cfg.shard
    streams = []
    for c in range(cfg.nc):
        m = shard_of == c
        s_c, d_c = src[m].astype(np.int64), dst[m].astype(np.int64)
        srcrow = (s_c // cfg.shard) * cfg.tp + (s_c % cfg.shard)
        q = srcrow // cfg.chunk
        dstloc = d_c - c * cfg.shard
        o = np.lexsort((dstloc, q))
        streams.append((srcrow[o], q[o], dstloc[o]))

    # canonical chunk lengths (max over cores, tile-rounded)
    chunk_lens = []
    for cc in range(cfg.n_chunks):
        L = max(int((q == cc).sum()) for _, q, _ in streams)
        chunk_lens.append(-(-L // P) * P)
    total_tiles = sum(chunk_lens) // P
    tslots = total_tiles * P

    # per-core slot streams
    PADV = 10 ** 9
    gz = np.zeros((cfg.nc, tslots), np.int32)
    dl = np.full((cfg.nc, tslots), PADV, np.int64)
    zero_local = cfg.sp                       # row sp of shard 0 of the chunk
    cbase = {}
    base = 0
    for cc in range(cfg.n_chunks):
        cbase[cc] = base
        for c in range(cfg.nc):
            srcrow, q, dstloc = streams[c]
            m = q == cc
            k = int(m.sum())
            gz[c, base:base + k] = srcrow[m] - cc * cfg.chunk
            gz[c, base + k:base + chunk_lens[cc]] = zero_local
            dl[c, base:base + k] = dstloc[m]
        base += chunk_lens[cc]

    # per-tile window sets (union over cores)
    W = dl // P
    tile_wins = []
    for t in range(total_tiles):
        wins = np.unique(W[:, t * P:(t + 1) * P])
        wins = sorted(int(w) for w in wins if w < cfg.t)
        assert len(wins) <= NSTREAM, (t, wins)
        tile_wins.append(wins)

    # bundles + events
    plan = []           # (chunk, btiles, goff, events, live_streams)
    first_seen = set()
    for cc in range(cfg.n_chunks):
        t0, t1 = cbase[cc] // P, (cbase[cc] + chunk_lens[cc]) // P
        tiles_of = {}
        for t in range(t0, t1):
            for j, w in enumerate(tile_wins[t]):
                tiles_of.setdefault(w, []).append((t, j))
        b0 = t0
        while b0 < t1:
            bt = min(cfg.max_span_tiles, t1 - b0)
            events = []
            live = set()
            for w in sorted(tiles_of):
                tl = [(t - b0, j) for (t, j) in tiles_of[w]
                      if b0 <= t < b0 + bt]
                if not tl:
                    continue
                ft = w not in first_seen
                first_seen.add(w)
                events.append((w, tl, ft))
                live.update(j for _, j in tl)
            plan.append((cc, bt, b0, events, sorted(live)))
            b0 += bt
    assert len(first_seen) == cfg.t, (len(first_seen), cfg.t)

    # dstw streams: value of slot relative to its tile's j-th window
    dws = np.full((NSTREAM, cfg.nc, tslots), 999.0, np.float64)
    tl_idx = np.arange(tslots) // P
    for j in range(NSTREAM):
        wj = np.array([tile_wins[t][j] if len(tile_wins[t]) > j else -10
                       for t in range(total_tiles)])
        v = dl - (wj[tl_idx] * P)[None, :]
        dws[j] = np.where((v >= 0) & (v < P) & (dl < PADV), v, 999.0)

    lo_hi = [(c * cfg.shard, (c + 1) * cfg.shard) for c in range(cfg.nc)]
    in_maps = []
    for c in range(cfg.nc):
        lo, hi = lo_hi[c]
        xT = np.zeros((cfg.in_feats, cfg.sp), BF)
        xT[:, :cfg.shard] = in_feat[lo:hi].T.astype(BF)
        full = np.ones(cfg.sp, np.float32)
        full[:cfg.shard] = dinv[lo:hi]
        dpm = np.ascontiguousarray(full.reshape(cfg.t, P).T)
        im = {
            "xT": xT, "dinv_pm": dpm,
            "gidx": _wrap16(gz[c].astype(np.int16)),
            "W1": np.asarray(W1, BF), "W2": np.asarray(W2, BF),
            "W3": np.asarray(W3, BF), "W4": np.asarray(W4, BF),
            "b1": np.asarray(b1, np.float32).reshape(-1, 1),
            "b2": np.asarray(b2, np.float32).reshape(-1, 1),
            "b3": np.asarray(b3, np.float32).reshape(-1, 1),
            "b4": np.asarray(b4, np.float32).reshape(-1, 1),
        }
        for j in range(NSTREAM):
            im[f"dstw{j}"] = np.ascontiguousarray(
                dws[j, c].reshape(total_tiles, P).T.astype(BF))
        in_maps.append(im)
    return in_maps, plan, total_tiles


# ---------------------------------------------------------------- builder

def build_nc(cfg, plan, total_tiles):
    H = cfg.h
    ST = cfg.max_span_tiles
    idx_cols = total_tiles * 8
    nc = bacc.Bacc("TRN2", target_bir_lowering=False, debug=False,
                   num_devices=cfg.nc)
    xT_d = nc.dram_tensor("xT", [cfg.in_feats, cfg.sp], BF16, kind="ExternalInput")
    dinv_d = nc.dram_tensor("dinv_pm", [P, cfg.t], F32, kind="ExternalInput")
    gidx_d = nc.dram_tensor("gidx", [P, idx_cols], I16, kind="ExternalInput")
    dstw_d = [nc.dram_tensor(f"dstw{j}", [P, total_tiles], BF16,
                             kind="ExternalInput") for j in range(NSTREAM)]
    W_d = {w: nc.dram_tensor(w, [cfg.in_feats if w in ("W1", "W4") else H, H],
                             BF16, kind="ExternalInput")
           for w in ("W1", "W2", "W3", "W4")}
    b_d = {b: nc.dram_tensor(b, [H, 1], F32, kind="ExternalInput")
           for b in ("b1", "b2", "b3", "b4")}
    outl_d = nc.dram_tensor("out_l", [H, cfg.sp], F32, kind="ExternalOutput")
    outh_d = nc.dram_tensor("out_h", [H, cfg.sp], F32, kind="ExternalOutput")

    relu = mybir.ActivationFunctionType.Relu
    cp = mybir.ActivationFunctionType.Copy

    with tile.TileContext(nc) as tc, ExitStack() as ctx:
        pers = ctx.enter_context(tc.tile_pool(name="pers", bufs=1))
        dram = ctx.enter_context(tc.tile_pool(name="dram", bufs=1, space="DRAM"))
        io = ctx.enter_context(tc.tile_pool(name="io", bufs=2))
        idxp = ctx.enter_context(tc.tile_pool(name="idxp", bufs=6))
        gbp = ctx.enter_context(tc.tile_pool(name="gbp", bufs=5))
        gbi = ctx.enter_context(tc.tile_pool(name="gbi", bufs=4))
        psum = ctx.enter_context(tc.tile_pool(name="psum", bufs=2, space="PSUM"))
        psum1 = ctx.enter_context(tc.tile_pool(name="psum1", bufs=2, space="PSUM"))
        psum2 = ctx.enter_context(tc.tile_pool(name="psum2", bufs=2, space="PSUM"))

        nc.gpsimd.load_library(mlp)

        f0 = pers.tile([P, cfg.t, 64], BF16, tag="f0")
        f1 = pers.tile([P, cfg.t, 64], BF16, tag="f1")
        f2 = pers.tile([P, cfg.t, 64], BF16, tag="f2")
        tbl = pers.tile([P, cfg.t + 1, 128], BF16, tag="tbl")
        dinv_s = pers.tile([P, cfg.t], F32, tag="dinv")
        Ws = {w: pers.tile([cfg.in_feats if w in ("W1", "W4") else H, H],
                           BF16, tag=w, name=w + "_s")
              for w in ("W1", "W2", "W3", "W4")}
        bs = {b: pers.tile([H, 1], F32, tag=b, name=b + "_s")
              for b in ("b1", "b2", "b3", "b4")}
        ident = pers.tile([P, P], BF16, tag="ident")
        sid3 = pers.tile([P, P], BF16, tag="sid3")
        sid075 = pers.tile([P, P], BF16, tag="sid075")
        sidm15 = pers.tile([P, P], BF16, tag="sidm15")

        tb_ins = [dram.tile([cfg.tp, 128], BF16, name=f"tb_in{r}")
                  for r in range(2)]
        tb_fulls = [dram.tile([cfg.tp * cfg.nc, 128], BF16, addr_space="Shared",
                              name=f"tb_full{r}") for r in range(2)]
        agg = pers.tile([P, cfg.t, 64], F32, tag="agg")
        iota_f = pers.tile([P, P], BF16, tag="iota_f")
        dinv_bf = pers.tile([P, cfg.t], BF16, tag="dinv_bf")

        for w in Ws:
            nc.sync.dma_start(Ws[w][:], W_d[w][:])
        for b in bs:
            nc.sync.dma_start(bs[b][:], b_d[b][:])
        nc.sync.dma_start(dinv_s[:], dinv_d[:])
        make_identity(nc, ident[:])
        nc.vector.tensor_scalar_mul(sid3[:], ident[:], 3.0)
        nc.vector.tensor_scalar_mul(sid075[:], ident[:], 0.75)
        nc.vector.tensor_scalar_mul(sidm15[:], ident[:], -1.5)
        nc.gpsimd.memset(tbl[:, cfg.t, :], 0.0)
        ioti = pers.tile([P, P], mybir.dt.int32, tag="ioti")
        nc.gpsimd.iota(ioti[:], pattern=[[1, P]], base=0, channel_multiplier=0)
        nc.vector.tensor_copy(iota_f[:], ioti[:])
        nc.vector.tensor_copy(dinv_bf[:], dinv_s[:])

        # ---- phase 1: MLP -> f0 node-major
        CH = cfg.mm_chunk
        for j0 in range(0, cfg.sp, CH):
            w = min(CH, cfg.sp - j0)
            xc = io.tile([cfg.in_feats, CH], BF16, tag="xc")
            nc.sync.dma_start(xc[:, :w], xT_d[:, j0:j0 + w])
            ps1 = psum.tile([H, CH], F32, tag="A")
            nc.tensor.matmul(ps1[:, :w], Ws["W1"][:], xc[:, :w],
                             start=True, stop=True)
            h1c = io.tile([H, CH], BF16, tag="h1c")
            nc.scalar.activation(h1c[:, :w], ps1[:, :w], relu, bias=bs["b1"][:])
            ps2 = psum.tile([H, CH], F32, tag="B")
            nc.tensor.matmul(ps2[:, :w], Ws["W2"][:], h1c[:, :w],
                             start=True, stop=True)
            h2c = io.tile([H, CH], BF16, tag="h2c")
            nc.scalar.activation(h2c[:, :w], ps2[:, :w], relu, bias=bs["b2"][:])
            for i in range(w // P):
                t = (j0 + i * P) // P
                ps3 = psum1.tile([P, 64], BF16, tag="C")
                nc.tensor.transpose(ps3[:], h2c[:, i * P:(i + 1) * P],
                                    ident[:H, :H])
                nc.scalar.activation(f0[:, t, :], ps3[:], cp)

        # ---- message passing rounds
        for rnd, (fprev, fnext) in enumerate([(f0, f1), (f1, f2)]):
            tb_in, tb_full = tb_ins[rnd], tb_fulls[rnd]
            nc.vector.tensor_tensor(
                tbl[:, :cfg.t, :64], fprev[:],
                dinv_bf[:, :, None].to_broadcast([P, cfg.t, 64]),
                mybir.AluOpType.mult)
            nc.sync.dma_start(
                tb_in[:].rearrange("(t p) f -> p t f", p=P), tbl[:])
            nc.gpsimd.collective_compute(
                "AllGather", mybir.AluOpType.bypass,
                replica_groups=[list(range(cfg.nc))],
                ins=[tb_in[:]], outs=[tb_full[:]])
            for (cc, btiles, goff, events, live) in plan:
                gi = idxp.tile([P, ST * 8], I16, tag="gi")
                nc.sync.dma_start(gi[:, :btiles * 8],
                                  gidx_d[:, goff * 8:(goff + btiles) * 8])
                dvs = {}
                for j in live:
                    dv = idxp.tile([P, ST], BF16, tag=f"dv{j}")
                    nc.sync.dma_start(dv[:, :btiles],
                                      dstw_d[j][:, goff:goff + btiles])
                    dvs[j] = dv
                gb = gbp.tile([P, ST, 128], BF16, tag="gb")
                ni = btiles * P
                nc.gpsimd.dma_gather(
                    gb[:, :btiles, :],
                    tb_full[cc * cfg.chunk:(cc + 1) * cfg.chunk, :],
                    gi[:, :btiles * 8], ni, ni, 128)
                for (ww, tl, ft) in events:
                    ind = gbi.tile([P, ST, P], BF16, tag="ind")
                    # build indicator planes in contiguous (j, t)-runs
                    i0 = 0
                    while i0 < len(tl):
                        t0, j0 = tl[i0]
                        i1 = i0 + 1
                        while (i1 < len(tl) and tl[i1][1] == j0
                               and tl[i1][0] == tl[i1 - 1][0] + 1):
                            i1 += 1
                        ln = i1 - i0
                        nc.vector.tensor_tensor(
                            ind[:, i0:i1, :],
                            iota_f[:, None, :].to_broadcast([P, ln, P]),
                            dvs[j0][:, t0:t0 + ln, None].to_broadcast(
                                [P, ln, P]),
                            mybir.AluOpType.is_equal)
                        i0 = i1
                    pw = psum2.tile([P, 64], F32, tag="D")
                    for i, (t, j) in enumerate(tl):
                        nc.tensor.matmul(pw[:], ind[:, i, :],
                                         gb[:, t, :64],
                                         start=(i == 0), stop=(i == len(tl) - 1))
                    if ft:
                        nc.vector.tensor_copy(agg[:, ww, :], pw[:])
                    else:
                        nc.vector.tensor_tensor(agg[:, ww, :], agg[:, ww, :],
                                                pw[:], mybir.AluOpType.add)
            # sliced per 4-window group so filter chunks can start while
            # late gather bundles are still in flight
            for c4 in range(0, cfg.t, 4):
                hi4 = min(c4 + 4, cfg.t)
                w4 = hi4 - c4
                nc.vector.tensor_tensor(
                    fnext[:, c4:hi4, :], agg[:, c4:hi4, :],
                    dinv_s[:, c4:hi4, None].to_broadcast([P, w4, 64]),
                    mybir.AluOpType.mult)
                nc.vector.tensor_tensor(fnext[:, c4:hi4, :],
                                        fprev[:, c4:hi4, :],
                                        fnext[:, c4:hi4, :],
                                        mybir.AluOpType.subtract)

        # ---- filters + output MLPs
        nc.vector.tensor_tensor(f0[:], f0[:], f1[:], mybir.AluOpType.subtract)
        for j0 in range(0, cfg.sp, CH):
            w = min(CH, cfg.sp - j0)
            zl = psum.tile([H, CH], F32, tag="A")
            z1 = psum.tile([H, CH], F32, tag="B")
            z2 = psum1.tile([H, CH], F32, tag="C")
            for i in range(w // P):
                t = (j0 + i * P) // P
                cs = slice(i * P, (i + 1) * P)
                nc.tensor.matmul(zl[:, cs], f0[:, t, :], sid3[:],
                                 start=True, stop=False)
                nc.tensor.matmul(zl[:, cs], f2[:, t, :], sid075[:],
                                 start=False, stop=True)
                nc.tensor.matmul(z1[:, cs], f1[:, t, :], sid3[:],
                                 start=True, stop=False)
                nc.tensor.matmul(z1[:, cs], f2[:, t, :], sidm15[:],
                                 start=False, stop=True)
                nc.tensor.matmul(z2[:, cs], f2[:, t, :], sid075[:],
                                 start=True, stop=True)
            zlc = io.tile([H, CH], BF16, tag="zlc")
            zhc = io.tile([P, CH], BF16, tag="zhc")
            nc.scalar.activation(zlc[:, :w], zl[:, :w], cp)
            nc.scalar.activation(zhc[:H, :w], z1[:, :w], cp)
            nc.scalar.activation(zhc[H:, :w], z2[:, :w], cp)
            pl = psum1.tile([H, CH], F32, tag="C")
            ph = psum.tile([H, CH], F32, tag="A")
            nc.tensor.matmul(pl[:, :w], Ws["W3"][:], zlc[:, :w],
                             start=True, stop=True)
            nc.tensor.matmul(ph[:, :w], Ws["W4"][:], zhc[:, :w],
                             start=True, stop=True)
            ol = io.tile([H, CH], F32, tag="ol")
            oh = io.tile([H, CH], F32, tag="oh")
            nc.scalar.activation(ol[:, :w], pl[:, :w], relu, bias=bs["b3"][:])
            nc.scalar.activation(oh[:, :w], ph[:, :w], relu, bias=bs["b4"][:])
            nc.sync.dma_start(outl_d[:, j0:j0 + w], ol[:, :w])
            nc.sync.dma_start(outh_d[:, j0:j0 + w], oh[:, :w])

    nc.compile()
    return nc


# ---------------------------------------------------------------- driver

_CACHE = {}


def run(cfg, inputs, run_fn=None, **spmd_kwargs):
    in_maps, plan, total_tiles = preprocess(cfg, **inputs)
    key = (cfg.n_nodes, cfg.n_edges, cfg.nc, cfg.max_span_tiles,
           total_tiles, repr(plan))
    if key not in _CACHE:
        _CACHE[key] = build_nc(cfg, plan, total_tiles)
    nc = _CACHE[key]
    if run_fn is not None:
        results = run_fn(nc, in_maps)
        res = None
    else:
        res = run_bass_kernel_spmd(nc, in_maps, core_ids=list(range(cfg.nc)),
                                   **spmd_kwargs)
        results = res.results
    h_l = np.zeros((cfg.n_nodes, cfg.h), np.float32)
    h_h = np.zeros((cfg.n_nodes, cfg.h), np.float32)
    for c in range(cfg.nc):
        lo = c * cfg.shard
        h_l[lo:lo + cfg.shard] = results[c]["out_l"].T[:cfg.shard]
        h_h[lo:lo + cfg.shard] = results[c]["out_h"].T[:cfg.shard]
    return h_l, h_h, res


def kernel(in_feat, src, dst, W1, b1, W2, b2, W3, b3, W4, b4):
    cfg = Cfg(100000, 1600000, 128, 64, 8)
    h_l, h_h, _ = run(cfg, dict(
        in_feat=np.asarray(in_feat, np.float32),
        src=np.asarray(src, np.int64), dst=np.asarray(dst, np.int64),
        W1=np.asarray(W1, np.float32), b1=np.asarray(b1, np.float32),
        W2=np.asarray(W2, np.float32), b2=np.asarray(b2, np.float32),
        W3=np.asarray(W3, np.float32), b3=np.asarray(b3, np.float32),
        W4=np.asarray(W4, np.float32), b4=np.asarray(b4, np.float32)))
    return h_l, h_h


# revision 19
# speedup vs baseline: 1.3521x; 1.0560x over previous
"""BWGNN (Bernstein-wavelet GNN) Trainium2 kernel, 8-core SPMD.

Sharding: nodes split 8 ways (graph/data parallel); edges partitioned by dst
shard; tiny weights replicated.  Per round of Laplacian message passing the
node-state table (dinv * f, bf16, rows padded to 256B) is distributed with
AllGather, then per-edge src rows are fetched with dma_gather (int16
indices, 1024 per instruction).

Gather descriptor generation on the Pool engine (~7.9ns/idx ucode) is the
hard bottleneck, so everything else hides under it:
- edge-slot streams are padded only per chunk (max over cores, ~0.7%);
  windows float: each canonical tile carries up to 4 candidate dst windows
  (union over cores), turned into one-hot indicators on the VectorEngine
  (iota is_eq vs 4 host bf16 dstw streams, self-masking), accumulated per
  window event on the TensorEngine in PSUM, first-touch-copied into an
  SBUF fp32 agg (no memset).
- the table is split in two parts (dst windows 0-47 / 48-97+zero row) with
  separate AllGathers, so part 0's collective overlaps the previous
  round's compute; bundles are interleaved across the 4 (part, core-half)
  chunks so windows complete progressively, and per 4-window group the
  next round's table mult + DMA (and in the last round the Bernstein
  filters + output MLP + output DMA) are emitted in-round.
All matmul operands are bf16 (PSUM fp32; agg, dinv, biases, outputs fp32).
"""

import sys
from contextlib import ExitStack

import numpy as np
import ml_dtypes

try:
    import concourse  # noqa: F401
except ImportError:  # pragma: no cover
    sys.path.insert(0, "/opt/trn_rl_repo")

import concourse.bacc as bacc
import concourse.bass as bass
import concourse.mybir as mybir
import concourse.tile as tile
from concourse.bass_utils import run_bass_kernel_spmd
from concourse.library_config import mlp
from concourse.masks import make_identity

P = 128
F32 = mybir.dt.float32
BF16 = mybir.dt.bfloat16
I16 = mybir.dt.int16
BF = ml_dtypes.bfloat16
NSTREAM = 4
GW = 4                    # windows per group (= 512 cols)


class Cfg:
    def __init__(self, n_nodes, n_edges, in_feats, h_feats, n_cores,
                 max_span_tiles=8, mm_chunk=512):
        assert n_nodes % n_cores == 0
        self.n_nodes, self.n_edges = n_nodes, n_edges
        self.in_feats, self.h = in_feats, h_feats
        self.nc = n_cores
        self.shard = n_nodes // n_cores
        self.sp = ((self.shard + P - 1) // P) * P      # padded shard 12544
        self.t = self.sp // P                          # node tiles 98
        self.tp = self.sp + P                          # table rows/core 12672
        # part split: windows [0, wsplit) / [wsplit, t) + zero tile
        self.wsplit = (self.t // 2 // GW) * GW         # 48
        self.p0_rows = self.wsplit * P                 # 6144 per core
        self.p1_rows = self.tp - self.p0_rows          # 6528 per core
        self.zero_off = self.sp - self.p0_rows         # 6400 (in part 1)
        # 4 chunks: (part, core-half)
        self.half = n_cores // 2
        self.chunks = [self.half * self.p0_rows, self.half * self.p0_rows,
                       self.half * self.p1_rows, self.half * self.p1_rows]
        assert all(c <= 32640 for c in self.chunks), self.chunks
        self.n_groups = -(-self.t // GW)               # 25
        self.max_span_tiles = max_span_tiles
        self.mm_chunk = mm_chunk
        assert mm_chunk == GW * P


# ---------------------------------------------------------------- host prep

def _wrap16(x):
    """flat int16 stream -> [128, n/16]: storage[p, col] = x[col*16 + p%16]."""
    assert len(x) % 16 == 0
    return np.tile(x.reshape(-1, 16).T, (8, 1)).copy()


def preprocess(cfg, in_feat, src, dst, W1, b1, W2, b2, W3, b3, W4, b4):
    n = cfg.n_nodes
    deg = np.bincount(dst, minlength=n).astype(np.float32)
    dinv = np.clip(deg, 1.0, None) ** -0.5

    shard_of = dst // cfg.shard
    streams = []
    for c in range(cfg.nc):
        m = shard_of == c
        s_c, d_c = src[m].astype(np.int64), dst[m].astype(np.int64)
        score = s_c // cfg.shard                       # owner core of src
        r = s_c % cfg.shard                            # local node row
        part = (r >= cfg.p0_rows).astype(np.int64)
        # chunk-local row within the part table
        row = np.where(part == 0,
                       (score % cfg.half) * cfg.p0_rows + r,
                       (score % cfg.half) * cfg.p1_rows + (r - cfg.p0_rows))
        q = part * 2 + score // cfg.half               # chunk 0..3
        dstloc = d_c - c * cfg.shard
        o = np.lexsort((dstloc, q))
        streams.append((row[o], q[o], dstloc[o]))

    # canonical chunk lengths (max over cores, tile-rounded)
    chunk_lens = []
    for cc in range(4):
        L = max(int((q == cc).sum()) for _, q, _ in streams)
        chunk_lens.append(-(-L // P) * P)
    total_tiles = sum(chunk_lens) // P
    tslots = total_tiles * P

    # per-core slot streams (slot order: chunk-major in chunk index order)
    PADV = 10 ** 9
    pad_row = [0, 0, cfg.zero_off, cfg.zero_off]       # masked anyway; p1 zero
    gz = np.zeros((cfg.nc, tslots), np.int32)
    dl = np.full((cfg.nc, tslots), PADV, np.int64)
    cbase = {}
    base = 0
    for cc in range(4):
        cbase[cc] = base
        for c in range(cfg.nc):
            row, q, dstloc = streams[c]
            m = q == cc
            k = int(m.sum())
            gz[c, base:base + k] = row[m]
            gz[c, base + k:base + chunk_lens[cc]] = pad_row[cc]
            dl[c, base:base + k] = dstloc[m]
        base += chunk_lens[cc]

    # per-tile window sets (union over cores)
    W = dl // P
    tile_wins = []
    for t in range(total_tiles):
        wins = np.unique(W[:, t * P:(t + 1) * P])
        wins = sorted(int(w) for w in wins if w < cfg.t)
        assert len(wins) <= NSTREAM, (t, wins)
        tile_wins.append(wins)

    # per-chunk bundle lists
    cb = []
    for cc in range(4):
        t0, t1 = cbase[cc] // P, (cbase[cc] + chunk_lens[cc]) // P
        tiles_of = {}
        for t in range(t0, t1):
            for j, w in enumerate(tile_wins[t]):
                tiles_of.setdefault(w, []).append((t, j))
        bl = []
        b0 = t0
        while b0 < t1:
            bt = min(cfg.max_span_tiles, t1 - b0)
            evs = []
            for w in sorted(tiles_of):
                tl = [(t - b0, j) for (t, j) in tiles_of[w]
                      if b0 <= t < b0 + bt]
                if tl:
                    evs.append((w, tl))
            bl.append((cc, bt, b0, evs))
            b0 += bt
        cb.append(bl)

    # emission order: p0-only prefix, then round-robin over all 4 chunks
    order = []
    ptr = [0, 0, 0, 0]
    PFX = 6
    for _ in range(PFX):
        for cc in (0, 1):
            if ptr[cc] < len(cb[cc]):
                order.append(cb[cc][ptr[cc]]); ptr[cc] += 1
    while any(ptr[cc] < len(cb[cc]) for cc in range(4)):
        for cc in range(4):
            if ptr[cc] < len(cb[cc]):
                order.append(cb[cc][ptr[cc]]); ptr[cc] += 1

    # first-touch + last-touch in emission order
    first_seen = set()
    last_touch = {}
    plan = []
    for bi, (cc, bt, b0, evs) in enumerate(order):
        events = []
        for (w, tl) in evs:
            ft = w not in first_seen
            first_seen.add(w)
            last_touch[w] = bi
            events.append((w, tl, ft))
        plan.append([cc, bt, b0, events, []])
    assert len(first_seen) == cfg.t, (len(first_seen), cfg.t)
    # group completion markers
    for g in range(cfg.n_groups):
        wlo, whi = g * GW, min((g + 1) * GW, cfg.t)
        done_at = max(last_touch[w] for w in range(wlo, whi))
        plan[done_at][4].append(g)

    # dstw streams: value of slot relative to its tile's j-th window
    dws = np.full((NSTREAM, cfg.nc, tslots), 999.0, np.float64)
    tl_idx = np.arange(tslots) // P
    for j in range(NSTREAM):
        wj = np.array([tile_wins[t][j] if len(tile_wins[t]) > j else -10
                       for t in range(total_tiles)])
        v = dl - (wj[tl_idx] * P)[None, :]
        dws[j] = np.where((v >= 0) & (v < P) & (dl < PADV), v, 999.0)

    in_maps = []
    for c in range(cfg.nc):
        lo, hi = c * cfg.shard, (c + 1) * cfg.shard
        xT = np.zeros((cfg.in_feats, cfg.sp), BF)
        xT[:, :cfg.shard] = in_feat[lo:hi].T.astype(BF)
        full = np.ones(cfg.sp, np.float32)
        full[:cfg.shard] = dinv[lo:hi]
        dpm = np.ascontiguousarray(full.reshape(cfg.t, P).T)
        # dstw_all [P, NSTREAM, total_tiles]
        da = np.stack([
            np.ascontiguousarray(dws[j, c].reshape(total_tiles, P).T)
            for j in range(NSTREAM)], axis=1).astype(BF)
        in_maps.append({
            "xT": xT, "dinv_pm": dpm,
            "gidx": _wrap16(gz[c].astype(np.int16)),
            "dstw": np.ascontiguousarray(da),
            "W1": np.asarray(W1, BF), "W2": np.asarray(W2, BF),
            "W3": np.asarray(W3, BF), "W4": np.asarray(W4, BF),
            "b1": np.asarray(b1, np.float32).reshape(-1, 1),
            "b2": np.asarray(b2, np.float32).reshape(-1, 1),
            "b3": np.asarray(b3, np.float32).reshape(-1, 1),
            "b4": np.asarray(b4, np.float32).reshape(-1, 1),
        })
    return in_maps, plan, total_tiles


# ---------------------------------------------------------------- builder

def build_nc(cfg, plan, total_tiles):
    H = cfg.h
    ST = cfg.max_span_tiles
    idx_cols = total_tiles * 8
    nc = bacc.Bacc("TRN2", target_bir_lowering=False, debug=False,
                   num_devices=cfg.nc)
    xT_d = nc.dram_tensor("xT", [cfg.in_feats, cfg.sp], BF16, kind="ExternalInput")
    dinv_d = nc.dram_tensor("dinv_pm", [P, cfg.t], F32, kind="ExternalInput")
    gidx_d = nc.dram_tensor("gidx", [P, idx_cols], I16, kind="ExternalInput")
    dstw_d = nc.dram_tensor("dstw", [P, NSTREAM, total_tiles], BF16,
                            kind="ExternalInput")
    W_d = {w: nc.dram_tensor(w, [cfg.in_feats if w in ("W1", "W4") else H, H],
                             BF16, kind="ExternalInput")
           for w in ("W1", "W2", "W3", "W4")}
    b_d = {b: nc.dram_tensor(b, [H, 1], F32, kind="ExternalInput")
           for b in ("b1", "b2", "b3", "b4")}
    outl_d = nc.dram_tensor("out_l", [H, cfg.sp], F32, kind="ExternalOutput")
    outh_d = nc.dram_tensor("out_h", [H, cfg.sp], F32, kind="ExternalOutput")

    relu = mybir.ActivationFunctionType.Relu
    cp = mybir.ActivationFunctionType.Copy

    with tile.TileContext(nc) as tc, ExitStack() as ctx:
        pers = ctx.enter_context(tc.tile_pool(name="pers", bufs=1))
        dram = ctx.enter_context(tc.tile_pool(name="dram", bufs=1, space="DRAM"))
        io = ctx.enter_context(tc.tile_pool(name="io", bufs=2))
        idxp = ctx.enter_context(tc.tile_pool(name="idxp", bufs=6))
        gbp = ctx.enter_context(tc.tile_pool(name="gbp", bufs=5))
        gbi = ctx.enter_context(tc.tile_pool(name="gbi", bufs=4))
        psum = ctx.enter_context(tc.tile_pool(name="psum", bufs=2, space="PSUM"))
        psum1 = ctx.enter_context(tc.tile_pool(name="psum1", bufs=2, space="PSUM"))
        psum2 = ctx.enter_context(tc.tile_pool(name="psum2", bufs=2, space="PSUM"))

        nc.gpsimd.load_library(mlp)

        f0 = pers.tile([P, cfg.t, 64], BF16, tag="f0")
        f1 = pers.tile([P, cfg.t, 64], BF16, tag="f1")
        f2 = pers.tile([P, cfg.t, 64], BF16, tag="f2")
        tbl = pers.tile([P, cfg.t + 1, 128], BF16, tag="tbl")
        dinv_s = pers.tile([P, cfg.t], F32, tag="dinv")
        dinv_bf = pers.tile([P, cfg.t], BF16, tag="dinv_bf")
        Ws = {w: pers.tile([cfg.in_feats if w in ("W1", "W4") else H, H],
                           BF16, tag=w, name=w + "_s")
              for w in ("W1", "W2", "W3", "W4")}
        bs = {b: pers.tile([H, 1], F32, tag=b, name=b + "_s")
              for b in ("b1", "b2", "b3", "b4")}
        ident = pers.tile([P, P], BF16, tag="ident")
        sid3 = pers.tile([P, P], BF16, tag="sid3")
        sid075 = pers.tile([P, P], BF16, tag="sid075")
        sidm15 = pers.tile([P, P], BF16, tag="sidm15")
        agg = pers.tile([P, cfg.t, 64], F32, tag="agg")
        iota_f = pers.tile([P, P], BF16, tag="iota_f")

        # DRAM tables: per round, two parts
        tb_in = [[dram.tile([cfg.p0_rows if p == 0 else cfg.p1_rows, 128],
                            BF16, name=f"tb_in{r}p{p}") for p in range(2)]
                 for r in range(2)]
        tb_full = [[dram.tile([(cfg.p0_rows if p == 0 else cfg.p1_rows)
                               * cfg.nc, 128], BF16, addr_space="Shared",
                              name=f"tb_full{r}p{p}") for p in range(2)]
                   for r in range(2)]

        for w in Ws:
            nc.sync.dma_start(Ws[w][:], W_d[w][:])
        for b in bs:
            nc.sync.dma_start(bs[b][:], b_d[b][:])
        nc.sync.dma_start(dinv_s[:], dinv_d[:])
        make_identity(nc, ident[:])
        nc.vector.tensor_scalar_mul(sid3[:], ident[:], 3.0)
        nc.vector.tensor_scalar_mul(sid075[:], ident[:], 0.75)
        nc.vector.tensor_scalar_mul(sidm15[:], ident[:], -1.5)
        nc.gpsimd.memset(tbl[:, cfg.t, :], 0.0)
        ioti = pers.tile([P, P], mybir.dt.int32, tag="ioti")
        nc.gpsimd.iota(ioti[:], pattern=[[1, P]], base=0, channel_multiplier=0)
        nc.vector.tensor_copy(iota_f[:], ioti[:])
        nc.vector.tensor_copy(dinv_bf[:], dinv_s[:])
        # zero rows of both rounds' part-1 tables
        for r in range(2):
            nc.sync.dma_start(
                tb_in[r][1][cfg.zero_off:cfg.zero_off + P, :]
                .rearrange("(t p) f -> p t f", p=P),
                tbl[:, cfg.t:cfg.t + 1, :])

        def table_group(rnd, fsrc, g):
            """tbl[:, grp] = dinv*f; DMA to tb_in[rnd]; collectives at marks."""
            wlo = g * GW
            whi = min(wlo + GW, cfg.t)
            nw = whi - wlo
            nc.vector.tensor_tensor(
                tbl[:, wlo:whi, :64], fsrc[:, wlo:whi, :],
                dinv_bf[:, wlo:whi, None].to_broadcast([P, nw, 64]),
                mybir.AluOpType.mult)
            p = 0 if whi <= cfg.wsplit else 1
            off = wlo * P - (0 if p == 0 else cfg.p0_rows)
            nc.sync.dma_start(
                tb_in[rnd][p][off:off + nw * P, :]
                .rearrange("(t p) f -> p t f", p=P),
                tbl[:, wlo:whi, :])

        def collective(rnd, p):
            nc.gpsimd.collective_compute(
                "AllGather", mybir.AluOpType.bypass,
                replica_groups=[list(range(cfg.nc))],
                ins=[tb_in[rnd][p][:]], outs=[tb_full[rnd][p][:]])

        # chunk -> (tb_full part, base row within part table)
        def chunk_region(rnd, cc):
            p = cc // 2
            half = cc % 2
            rows = cfg.p0_rows if p == 0 else cfg.p1_rows
            base = half * cfg.half * rows
            return tb_full[rnd][p], base, cfg.half * rows

        gsplit = cfg.wsplit // GW                      # groups in part 0
        # bundle index after which all part-0 groups are complete
        seen_g = set()
        p0_fire = None
        for bi, (_, _, _, _, gdone) in enumerate(plan):
            seen_g.update(gdone)
            if p0_fire is None and all(g in seen_g for g in range(gsplit)):
                p0_fire = bi
                break
        assert p0_fire is not None

        # ---- phase 1: MLP -> f0 node-major; table 0 produced per group
        CH = cfg.mm_chunk
        n_mlp = -(-cfg.sp // CH)
        for k in range(n_mlp):
            j0 = k * CH
            w = min(CH, cfg.sp - j0)
            xc = io.tile([cfg.in_feats, CH], BF16, tag="xc")
            nc.sync.dma_start(xc[:, :w], xT_d[:, j0:j0 + w])
            ps1 = psum.tile([H, CH], F32, tag="A")
            nc.tensor.matmul(ps1[:, :w], Ws["W1"][:], xc[:, :w],
                             start=True, stop=True)
            h1c = io.tile([H, CH], BF16, tag="h1c")
            nc.scalar.activation(h1c[:, :w], ps1[:, :w], relu, bias=bs["b1"][:])
            ps2 = psum.tile([H, CH], F32, tag="B")
            nc.tensor.matmul(ps2[:, :w], Ws["W2"][:], h1c[:, :w],
                             start=True, stop=True)
            h2c = io.tile([H, CH], BF16, tag="h2c")
            nc.scalar.activation(h2c[:, :w], ps2[:, :w], relu, bias=bs["b2"][:])
            for i in range(w // P):
                t = (j0 + i * P) // P
                ps3 = psum1.tile([P, 64], BF16, tag="C")
                nc.tensor.transpose(ps3[:], h2c[:, i * P:(i + 1) * P],
                                    ident[:H, :H])
                nc.scalar.activation(f0[:, t, :], ps3[:], cp)
            table_group(0, f0, k)
            if k == gsplit - 1:
                collective(0, 0)
        collective(0, 1)

        # ---- message passing rounds
        for rnd, (fprev, fnext) in enumerate([(f0, f1), (f1, f2)]):
            for bi, (cc, btiles, goff, events, gdone) in enumerate(plan):
                tbf, cbase_, clen = chunk_region(rnd, cc)
                gi = idxp.tile([P, ST * 8], I16, tag="gi")
                nc.scalar.dma_start(gi[:, :btiles * 8],
                                    gidx_d[:, goff * 8:(goff + btiles) * 8])
                dv = idxp.tile([P, NSTREAM, ST], BF16, tag="dv")
                nc.scalar.dma_start(dv[:, :, :btiles],
                                    dstw_d[:, :, goff:goff + btiles])
                gb = gbp.tile([P, ST, 128], BF16, tag="gb")
                ni = btiles * P
                nc.gpsimd.dma_gather(
                    gb[:, :btiles, :], tbf[cbase_:cbase_ + clen, :],
                    gi[:, :btiles * 8], ni, ni, 128)
                for (ww, tl, ft) in events:
                    ind = gbi.tile([P, ST, P], BF16, tag="ind")
                    i0 = 0
                    while i0 < len(tl):
                        t0, j0_ = tl[i0]
                        i1 = i0 + 1
                        while (i1 < len(tl) and tl[i1][1] == j0_
                               and tl[i1][0] == tl[i1 - 1][0] + 1):
                            i1 += 1
                        ln = i1 - i0
                        nc.vector.tensor_tensor(
                            ind[:, i0:i1, :],
                            iota_f[:, None, :].to_broadcast([P, ln, P]),
                            dv[:, j0_, t0:t0 + ln, None].to_broadcast(
                                [P, ln, P]),
                            mybir.AluOpType.is_equal)
                        i0 = i1
                    pw = psum2.tile([P, 64], F32, tag="D")
                    for i, (t, j) in enumerate(tl):
                        nc.tensor.matmul(pw[:], ind[:, i, :],
                                         gb[:, t, :64],
                                         start=(i == 0), stop=(i == len(tl) - 1))
                    if ft:
                        nc.vector.tensor_copy(agg[:, ww, :], pw[:])
                    else:
                        nc.vector.tensor_tensor(agg[:, ww, :], agg[:, ww, :],
                                                pw[:], mybir.AluOpType.add)
                for g in gdone:
                    wlo = g * GW
                    whi = min(wlo + GW, cfg.t)
                    nw = whi - wlo
                    # fnext = fprev - dinv*agg
                    nc.vector.tensor_tensor(
                        fnext[:, wlo:whi, :], agg[:, wlo:whi, :],
                        dinv_s[:, wlo:whi, None].to_broadcast([P, nw, 64]),
                        mybir.AluOpType.mult)
                    nc.vector.tensor_tensor(
                        fnext[:, wlo:whi, :], fprev[:, wlo:whi, :],
                        fnext[:, wlo:whi, :], mybir.AluOpType.subtract)
                    if rnd == 0:
                        # f0 := f0 - f1 (filters); round-2 table from f1
                        nc.vector.tensor_tensor(
                            f0[:, wlo:whi, :], f0[:, wlo:whi, :],
                            f1[:, wlo:whi, :], mybir.AluOpType.subtract)
                        table_group(1, f1, g)
                    else:
                        # filters + output MLP for this group
                        j0 = wlo * P
                        w = whi * P - j0
                        zl = psum.tile([H, CH], F32, tag="A")
                        z1 = psum.tile([H, CH], F32, tag="B")
                        z2 = psum1.tile([H, CH], F32, tag="C")
                        for i in range(nw):
                            t = wlo + i
                            cs = slice(i * P, (i + 1) * P)
                            nc.tensor.matmul(zl[:, cs], f0[:, t, :], sid3[:],
                                             start=True, stop=False)
                            nc.tensor.matmul(zl[:, cs], f2[:, t, :], sid075[:],
                                             start=False, stop=True)
                            nc.tensor.matmul(z1[:, cs], f1[:, t, :], sid3[:],
                                             start=True, stop=False)
                            nc.tensor.matmul(z1[:, cs], f2[:, t, :], sidm15[:],
                                             start=False, stop=True)
                            nc.tensor.matmul(z2[:, cs], f2[:, t, :], sid075[:],
                                             start=True, stop=True)
                        zlc = io.tile([H, CH], BF16, tag="zlc")
                        zhc = io.tile([P, CH], BF16, tag="zhc")
                        nc.scalar.activation(zlc[:, :w], zl[:, :w], cp)
                        nc.scalar.activation(zhc[:H, :w], z1[:, :w], cp)
                        nc.scalar.activation(zhc[H:, :w], z2[:, :w], cp)
                        pl = psum1.tile([H, CH], F32, tag="C")
                        ph = psum.tile([H, CH], F32, tag="A")
                        nc.tensor.matmul(pl[:, :w], Ws["W3"][:], zlc[:, :w],
                                         start=True, stop=True)
                        nc.tensor.matmul(ph[:, :w], Ws["W4"][:], zhc[:, :w],
                                         start=True, stop=True)
                        ol = io.tile([H, CH], F32, tag="ol")
                        oh = io.tile([H, CH], F32, tag="oh")
                        nc.scalar.activation(ol[:, :w], pl[:, :w], relu,
                                             bias=bs["b3"][:])
                        nc.scalar.activation(oh[:, :w], ph[:, :w], relu,
                                             bias=bs["b4"][:])
                        nc.sync.dma_start(outl_d[:, j0:j0 + w], ol[:, :w])
                        nc.sync.dma_start(outh_d[:, j0:j0 + w], oh[:, :w])
                if rnd == 0 and bi == p0_fire:
                    collective(1, 0)
            if rnd == 0:
                collective(1, 1)

    nc.compile()
    return nc


# ---------------------------------------------------------------- driver

_CACHE = {}


def run(cfg, inputs, run_fn=None, **spmd_kwargs):
    in_maps, plan, total_tiles = preprocess(cfg, **inputs)
    key = (cfg.n_nodes, cfg.n_edges, cfg.nc, cfg.max_span_tiles,
           total_tiles, repr(plan))
    if key not in _CACHE:
        _CACHE[key] = build_nc(cfg, plan, total_tiles)
    nc = _CACHE[key]
    if run_fn is not None:
        results = run_fn(nc, in_maps)
        res = None
    else:
        res = run_bass_kernel_spmd(nc, in_maps, core_ids=list(range(cfg.nc)),
                                   **spmd_kwargs)
        results = res.results
    h_l = np.zeros((cfg.n_nodes, cfg.h), np.float32)
    h_h = np.zeros((cfg.n_nodes, cfg.h), np.float32)
    for c in range(cfg.nc):
        lo = c * cfg.shard
        h_l[lo:lo + cfg.shard] = results[c]["out_l"].T[:cfg.shard]
        h_h[lo:lo + cfg.shard] = results[c]["out_h"].T[:cfg.shard]
    return h_l, h_h, res


def kernel(in_feat, src, dst, W1, b1, W2, b2, W3, b3, W4, b4):
    cfg = Cfg(100000, 1600000, 128, 64, 8)
    h_l, h_h, _ = run(cfg, dict(
        in_feat=np.asarray(in_feat, np.float32),
        src=np.asarray(src, np.int64), dst=np.asarray(dst, np.int64),
        W1=np.asarray(W1, np.float32), b1=np.asarray(b1, np.float32),
        W2=np.asarray(W2, np.float32), b2=np.asarray(b2, np.float32),
        W3=np.asarray(W3, np.float32), b3=np.asarray(b3, np.float32),
        W4=np.asarray(W4, np.float32), b4=np.asarray(b4, np.float32)))
    return h_l, h_h


# revision 21
# speedup vs baseline: 1.3757x; 1.0174x over previous
"""BWGNN (Bernstein-wavelet GNN) Trainium2 kernel, 8-core SPMD.

Sharding: nodes split 8 ways (graph/data parallel); edges partitioned by dst
shard; tiny weights replicated.  Per round of Laplacian message passing the
node-state table (dinv * f, bf16, rows padded to 256B) is distributed with
AllGather, then per-edge src rows are fetched with dma_gather (int16
indices, 1024 per instruction).

Gather descriptor generation on the Pool engine (~7.9ns/idx ucode) is the
hard bottleneck, so everything else hides under it:
- edge-slot streams are padded only per chunk (max over cores, ~0.7%);
  windows float: each canonical tile carries up to 4 candidate dst windows
  (union over cores), turned into one-hot indicators on the VectorEngine
  (iota is_eq vs 4 host bf16 dstw streams, self-masking), accumulated per
  window event on the TensorEngine in PSUM, first-touch-copied into an
  SBUF fp32 agg (no memset).
- the table is split in two parts (dst windows 0-47 / 48-97+zero row) with
  separate AllGathers, so part 0's collective overlaps the previous
  round's compute; bundles are interleaved across the 4 (part, core-half)
  chunks so windows complete progressively, and per 4-window group the
  next round's table mult + DMA (and in the last round the Bernstein
  filters + output MLP + output DMA) are emitted in-round.
All matmul operands are bf16 (PSUM fp32; agg, dinv, biases, outputs fp32).
"""

import sys
from contextlib import ExitStack

import numpy as np
import ml_dtypes

try:
    import concourse  # noqa: F401
except ImportError:  # pragma: no cover
    sys.path.insert(0, "/opt/trn_rl_repo")

import concourse.bacc as bacc
import concourse.bass as bass
import concourse.mybir as mybir
import concourse.tile as tile
from concourse.bass_utils import run_bass_kernel_spmd
from concourse.library_config import mlp
from concourse.masks import make_identity

P = 128
F32 = mybir.dt.float32
BF16 = mybir.dt.bfloat16
I16 = mybir.dt.int16
BF = ml_dtypes.bfloat16
NSTREAM = 4
GW = 4                    # windows per group (= 512 cols)


class Cfg:
    def __init__(self, n_nodes, n_edges, in_feats, h_feats, n_cores,
                 max_span_tiles=8, mm_chunk=512):
        assert n_nodes % n_cores == 0
        self.n_nodes, self.n_edges = n_nodes, n_edges
        self.in_feats, self.h = in_feats, h_feats
        self.nc = n_cores
        self.shard = n_nodes // n_cores
        self.sp = ((self.shard + P - 1) // P) * P      # padded shard 12544
        self.t = self.sp // P                          # node tiles 98
        self.tp = self.sp + P                          # table rows/core 12672
        # part split: windows [0, wsplit) / [wsplit, t) + zero tile
        self.wsplit = (self.t // 2 // GW) * GW         # 48
        self.p0_rows = self.wsplit * P                 # 6144 per core
        self.p1_rows = self.tp - self.p0_rows          # 6528 per core
        self.zero_off = self.sp - self.p0_rows         # 6400 (in part 1)
        # 4 chunks: (part, core-half)
        self.half = n_cores // 2
        self.chunks = [self.half * self.p0_rows, self.half * self.p0_rows,
                       self.half * self.p1_rows, self.half * self.p1_rows]
        assert all(c <= 32640 for c in self.chunks), self.chunks
        self.n_groups = -(-self.t // GW)               # 25
        self.max_span_tiles = max_span_tiles
        self.mm_chunk = mm_chunk
        assert mm_chunk == GW * P


# ---------------------------------------------------------------- host prep

def _wrap16(x):
    """flat int16 stream -> [128, n/16]: storage[p, col] = x[col*16 + p%16]."""
    assert len(x) % 16 == 0
    return np.tile(x.reshape(-1, 16).T, (8, 1)).copy()


def preprocess(cfg, in_feat, src, dst, W1, b1, W2, b2, W3, b3, W4, b4):
    n = cfg.n_nodes
    deg = np.bincount(dst, minlength=n).astype(np.float32)
    dinv = np.clip(deg, 1.0, None) ** -0.5

    shard_of = dst // cfg.shard
    streams = []
    for c in range(cfg.nc):
        m = shard_of == c
        s_c, d_c = src[m].astype(np.int64), dst[m].astype(np.int64)
        score = s_c // cfg.shard                       # owner core of src
        r = s_c % cfg.shard                            # local node row
        part = (r >= cfg.p0_rows).astype(np.int64)
        # chunk-local row within the part table
        row = np.where(part == 0,
                       (score % cfg.half) * cfg.p0_rows + r,
                       (score % cfg.half) * cfg.p1_rows + (r - cfg.p0_rows))
        q = part * 2 + score // cfg.half               # chunk 0..3
        dstloc = d_c - c * cfg.shard
        o = np.lexsort((dstloc, q))
        streams.append((row[o], q[o], dstloc[o]))

    # canonical chunk lengths (max over cores, tile-rounded)
    chunk_lens = []
    for cc in range(4):
        L = max(int((q == cc).sum()) for _, q, _ in streams)
        chunk_lens.append(-(-L // P) * P)
    total_tiles = sum(chunk_lens) // P
    tslots = total_tiles * P

    # per-core slot streams (slot order: chunk-major in chunk index order)
    PADV = 10 ** 9
    pad_row = [0, 0, cfg.zero_off, cfg.zero_off]       # masked anyway; p1 zero
    gz = np.zeros((cfg.nc, tslots), np.int32)
    dl = np.full((cfg.nc, tslots), PADV, np.int64)
    cbase = {}
    base = 0
    for cc in range(4):
        cbase[cc] = base
        for c in range(cfg.nc):
            row, q, dstloc = streams[c]
            m = q == cc
            k = int(m.sum())
            gz[c, base:base + k] = row[m]
            gz[c, base + k:base + chunk_lens[cc]] = pad_row[cc]
            dl[c, base:base + k] = dstloc[m]
        base += chunk_lens[cc]

    # per-tile window sets (union over cores)
    W = dl // P
    tile_wins = []
    for t in range(total_tiles):
        wins = np.unique(W[:, t * P:(t + 1) * P])
        wins = sorted(int(w) for w in wins if w < cfg.t)
        assert len(wins) <= NSTREAM, (t, wins)
        tile_wins.append(wins)

    # per-chunk bundle lists
    cb = []
    for cc in range(4):
        t0, t1 = cbase[cc] // P, (cbase[cc] + chunk_lens[cc]) // P
        tiles_of = {}
        for t in range(t0, t1):
            for j, w in enumerate(tile_wins[t]):
                tiles_of.setdefault(w, []).append((t, j))
        bl = []
        b0 = t0
        while b0 < t1:
            bt = min(cfg.max_span_tiles, t1 - b0)
            evs = []
            for w in sorted(tiles_of):
                tl = [(t - b0, j) for (t, j) in tiles_of[w]
                      if b0 <= t < b0 + bt]
                if tl:
                    evs.append((w, tl))
            bl.append((cc, bt, b0, evs))
            b0 += bt
        cb.append(bl)

    # emission order: p0-only prefix, then round-robin over all 4 chunks
    order = []
    ptr = [0, 0, 0, 0]
    PFX = 6
    for _ in range(PFX):
        for cc in (0, 1):
            if ptr[cc] < len(cb[cc]):
                order.append(cb[cc][ptr[cc]]); ptr[cc] += 1
    while any(ptr[cc] < len(cb[cc]) for cc in range(4)):
        for cc in range(4):
            if ptr[cc] < len(cb[cc]):
                order.append(cb[cc][ptr[cc]]); ptr[cc] += 1

    # first-touch + last-touch in emission order
    first_seen = set()
    last_touch = {}
    plan = []
    for bi, (cc, bt, b0, evs) in enumerate(order):
        events = []
        for (w, tl) in evs:
            ft = w not in first_seen
            first_seen.add(w)
            last_touch[w] = bi
            events.append((w, tl, ft))
        plan.append([cc, bt, b0, events, []])
    assert len(first_seen) == cfg.t, (len(first_seen), cfg.t)
    # group completion markers
    for g in range(cfg.n_groups):
        wlo, whi = g * GW, min((g + 1) * GW, cfg.t)
        done_at = max(last_touch[w] for w in range(wlo, whi))
        plan[done_at][4].append(g)

    # dstw streams: value of slot relative to its tile's j-th window
    dws = np.full((NSTREAM, cfg.nc, tslots), 999.0, np.float64)
    tl_idx = np.arange(tslots) // P
    for j in range(NSTREAM):
        wj = np.array([tile_wins[t][j] if len(tile_wins[t]) > j else -10
                       for t in range(total_tiles)])
        v = dl - (wj[tl_idx] * P)[None, :]
        dws[j] = np.where((v >= 0) & (v < P) & (dl < PADV), v, 999.0)

    in_maps = []
    for c in range(cfg.nc):
        lo, hi = c * cfg.shard, (c + 1) * cfg.shard
        xT = np.zeros((cfg.in_feats, cfg.sp), BF)
        xT[:, :cfg.shard] = in_feat[lo:hi].T.astype(BF)
        full = np.ones(cfg.sp, np.float32)
        full[:cfg.shard] = dinv[lo:hi]
        dpm = np.ascontiguousarray(full.reshape(cfg.t, P).T)
        # dstw_all [P, NSTREAM, total_tiles]
        da = np.stack([
            np.ascontiguousarray(dws[j, c].reshape(total_tiles, P).T)
            for j in range(NSTREAM)], axis=1).astype(BF)
        in_maps.append({
            "xT": xT, "dinv_pm": dpm,
            "gidx": _wrap16(gz[c].astype(np.int16)),
            "dstw": np.ascontiguousarray(da),
            "W1": np.asarray(W1, BF), "W2": np.asarray(W2, BF),
            "W3": np.asarray(W3, BF), "W4": np.asarray(W4, BF),
            "b1": np.asarray(b1, np.float32).reshape(-1, 1),
            "b2": np.asarray(b2, np.float32).reshape(-1, 1),
            "b3": np.asarray(b3, np.float32).reshape(-1, 1),
            "b4": np.asarray(b4, np.float32).reshape(-1, 1),
        })
    return in_maps, plan, total_tiles


# ---------------------------------------------------------------- builder

def build_nc(cfg, plan, total_tiles):
    H = cfg.h
    ST = cfg.max_span_tiles
    idx_cols = total_tiles * 8
    nc = bacc.Bacc("TRN2", target_bir_lowering=False, debug=False,
                   num_devices=cfg.nc)
    xT_d = nc.dram_tensor("xT", [cfg.in_feats, cfg.sp], BF16, kind="ExternalInput")
    dinv_d = nc.dram_tensor("dinv_pm", [P, cfg.t], F32, kind="ExternalInput")
    gidx_d = nc.dram_tensor("gidx", [P, idx_cols], I16, kind="ExternalInput")
    dstw_d = nc.dram_tensor("dstw", [P, NSTREAM, total_tiles], BF16,
                            kind="ExternalInput")
    W_d = {w: nc.dram_tensor(w, [cfg.in_feats if w in ("W1", "W4") else H, H],
                             BF16, kind="ExternalInput")
           for w in ("W1", "W2", "W3", "W4")}
    b_d = {b: nc.dram_tensor(b, [H, 1], F32, kind="ExternalInput")
           for b in ("b1", "b2", "b3", "b4")}
    outl_d = nc.dram_tensor("out_l", [H, cfg.sp], F32, kind="ExternalOutput")
    outh_d = nc.dram_tensor("out_h", [H, cfg.sp], F32, kind="ExternalOutput")

    relu = mybir.ActivationFunctionType.Relu
    cp = mybir.ActivationFunctionType.Copy

    with tile.TileContext(nc) as tc, ExitStack() as ctx:
        pers = ctx.enter_context(tc.tile_pool(name="pers", bufs=1))
        dram = ctx.enter_context(tc.tile_pool(name="dram", bufs=1, space="DRAM"))
        io = ctx.enter_context(tc.tile_pool(name="io", bufs=2))
        idxp = ctx.enter_context(tc.tile_pool(name="idxp", bufs=8))
        gbp = ctx.enter_context(tc.tile_pool(name="gbp", bufs=6))
        gbi = ctx.enter_context(tc.tile_pool(name="gbi", bufs=4))
        psum = ctx.enter_context(tc.tile_pool(name="psum", bufs=2, space="PSUM"))
        psum1 = ctx.enter_context(tc.tile_pool(name="psum1", bufs=2, space="PSUM"))
        psum2 = ctx.enter_context(tc.tile_pool(name="psum2", bufs=2, space="PSUM"))

        nc.gpsimd.load_library(mlp)

        f0 = pers.tile([P, cfg.t, 64], BF16, tag="f0")
        f1 = pers.tile([P, cfg.t, 64], BF16, tag="f1")
        f2 = pers.tile([P, cfg.t, 64], BF16, tag="f2")
        tbl = pers.tile([P, cfg.t + 1, 128], BF16, tag="tbl")
        dinv_s = pers.tile([P, cfg.t], F32, tag="dinv")
        dinv_bf = pers.tile([P, cfg.t], BF16, tag="dinv_bf")
        Ws = {w: pers.tile([cfg.in_feats if w in ("W1", "W4") else H, H],
                           BF16, tag=w, name=w + "_s")
              for w in ("W1", "W2", "W3", "W4")}
        bs = {b: pers.tile([H, 1], F32, tag=b, name=b + "_s")
              for b in ("b1", "b2", "b3", "b4")}
        ident = pers.tile([P, P], BF16, tag="ident")
        sid3 = pers.tile([P, P], BF16, tag="sid3")
        sid075 = pers.tile([P, P], BF16, tag="sid075")
        sidm15 = pers.tile([P, P], BF16, tag="sidm15")
        agg = pers.tile([P, cfg.t, 64], F32, tag="agg")
        iota_f = pers.tile([P, P], BF16, tag="iota_f")

        # DRAM tables: per round, two parts
        tb_in = [[dram.tile([cfg.p0_rows if p == 0 else cfg.p1_rows, 128],
                            BF16, name=f"tb_in{r}p{p}") for p in range(2)]
                 for r in range(2)]
        tb_full = [[dram.tile([(cfg.p0_rows if p == 0 else cfg.p1_rows)
                               * cfg.nc, 128], BF16, addr_space="Shared",
                              name=f"tb_full{r}p{p}") for p in range(2)]
                   for r in range(2)]

        for w in Ws:
            nc.sync.dma_start(Ws[w][:], W_d[w][:])
        for b in bs:
            nc.sync.dma_start(bs[b][:], b_d[b][:])
        nc.sync.dma_start(dinv_s[:], dinv_d[:])
        make_identity(nc, ident[:])
        nc.vector.tensor_scalar_mul(sid3[:], ident[:], 3.0)
        nc.vector.tensor_scalar_mul(sid075[:], ident[:], 0.75)
        nc.vector.tensor_scalar_mul(sidm15[:], ident[:], -1.5)
        nc.gpsimd.memset(tbl[:, cfg.t, :], 0.0)
        ioti = pers.tile([P, P], mybir.dt.int32, tag="ioti")
        nc.gpsimd.iota(ioti[:], pattern=[[1, P]], base=0, channel_multiplier=0)
        nc.vector.tensor_copy(iota_f[:], ioti[:])
        nc.vector.tensor_copy(dinv_bf[:], dinv_s[:])
        # zero rows of both rounds' part-1 tables
        for r in range(2):
            nc.sync.dma_start(
                tb_in[r][1][cfg.zero_off:cfg.zero_off + P, :]
                .rearrange("(t p) f -> p t f", p=P),
                tbl[:, cfg.t:cfg.t + 1, :])

        def table_group(rnd, fsrc, g):
            """tbl[:, grp] = dinv*f; DMA to tb_in[rnd]; collectives at marks."""
            wlo = g * GW
            whi = min(wlo + GW, cfg.t)
            nw = whi - wlo
            nc.vector.tensor_tensor(
                tbl[:, wlo:whi, :64], fsrc[:, wlo:whi, :],
                dinv_bf[:, wlo:whi, None].to_broadcast([P, nw, 64]),
                mybir.AluOpType.mult)
            p = 0 if whi <= cfg.wsplit else 1
            off = wlo * P - (0 if p == 0 else cfg.p0_rows)
            nc.sync.dma_start(
                tb_in[rnd][p][off:off + nw * P, :]
                .rearrange("(t p) f -> p t f", p=P),
                tbl[:, wlo:whi, :])

        def collective(rnd, p):
            nc.gpsimd.collective_compute(
                "AllGather", mybir.AluOpType.bypass,
                replica_groups=[list(range(cfg.nc))],
                ins=[tb_in[rnd][p][:]], outs=[tb_full[rnd][p][:]])

        # chunk -> (tb_full part, base row within part table)
        def chunk_region(rnd, cc):
            p = cc // 2
            half = cc % 2
            rows = cfg.p0_rows if p == 0 else cfg.p1_rows
            base = half * cfg.half * rows
            return tb_full[rnd][p], base, cfg.half * rows

        gsplit = cfg.wsplit // GW                      # groups in part 0
        # bundle index after which all part-0 groups are complete
        seen_g = set()
        p0_fire = None
        for bi, (_, _, _, _, gdone) in enumerate(plan):
            seen_g.update(gdone)
            if p0_fire is None and all(g in seen_g for g in range(gsplit)):
                p0_fire = bi
                break
        assert p0_fire is not None

        # ---- phase 1: MLP -> f0 node-major; table 0 produced per group
        CH = cfg.mm_chunk
        n_mlp = -(-cfg.sp // CH)
        for k in range(n_mlp):
            j0 = k * CH
            w = min(CH, cfg.sp - j0)
            xc = io.tile([cfg.in_feats, CH], BF16, tag="xc")
            nc.sync.dma_start(xc[:, :w], xT_d[:, j0:j0 + w])
            ps1 = psum.tile([H, CH], F32, tag="A")
            nc.tensor.matmul(ps1[:, :w], Ws["W1"][:], xc[:, :w],
                             start=True, stop=True)
            h1c = io.tile([H, CH], BF16, tag="h1c")
            nc.scalar.activation(h1c[:, :w], ps1[:, :w], relu, bias=bs["b1"][:])
            ps2 = psum.tile([H, CH], F32, tag="B")
            nc.tensor.matmul(ps2[:, :w], Ws["W2"][:], h1c[:, :w],
                             start=True, stop=True)
            h2c = io.tile([H, CH], BF16, tag="h2c")
            nc.scalar.activation(h2c[:, :w], ps2[:, :w], relu, bias=bs["b2"][:])
            for i in range(w // P):
                t = (j0 + i * P) // P
                ps3 = psum1.tile([P, 64], BF16, tag="C")
                nc.tensor.transpose(ps3[:], h2c[:, i * P:(i + 1) * P],
                                    ident[:H, :H])
                nc.scalar.activation(f0[:, t, :], ps3[:], cp)
            table_group(0, f0, k)
            if k == gsplit - 1:
                collective(0, 0)
        collective(0, 1)

        # ---- message passing rounds
        for rnd, (fprev, fnext) in enumerate([(f0, f1), (f1, f2)]):
            for bi, (cc, btiles, goff, events, gdone) in enumerate(plan):
                tbf, cbase_, clen = chunk_region(rnd, cc)
                gi = idxp.tile([P, ST * 8], I16, tag="gi")
                nc.sync.dma_start(gi[:, :btiles * 8],
                                  gidx_d[:, goff * 8:(goff + btiles) * 8])
                dv = idxp.tile([P, NSTREAM, ST], BF16, tag="dv")
                nc.scalar.dma_start(dv[:, :, :btiles],
                                    dstw_d[:, :, goff:goff + btiles])
                gb = gbp.tile([P, ST, 128], BF16, tag="gb")
                ni = btiles * P
                nc.gpsimd.dma_gather(
                    gb[:, :btiles, :], tbf[cbase_:cbase_ + clen, :],
                    gi[:, :btiles * 8], ni, ni, 128)
                for (ww, tl, ft) in events:
                    ind = gbi.tile([P, ST, P], BF16, tag="ind")
                    i0 = 0
                    while i0 < len(tl):
                        t0, j0_ = tl[i0]
                        i1 = i0 + 1
                        while (i1 < len(tl) and tl[i1][1] == j0_
                               and tl[i1][0] == tl[i1 - 1][0] + 1):
                            i1 += 1
                        ln = i1 - i0
                        nc.vector.tensor_tensor(
                            ind[:, i0:i1, :],
                            iota_f[:, None, :].to_broadcast([P, ln, P]),
                            dv[:, j0_, t0:t0 + ln, None].to_broadcast(
                                [P, ln, P]),
                            mybir.AluOpType.is_equal)
                        i0 = i1
                    pw = psum2.tile([P, 64], F32, tag="D")
                    for i, (t, j) in enumerate(tl):
                        nc.tensor.matmul(pw[:], ind[:, i, :],
                                         gb[:, t, :64],
                                         start=(i == 0), stop=(i == len(tl) - 1))
                    if ft:
                        nc.vector.tensor_copy(agg[:, ww, :], pw[:])
                    else:
                        nc.vector.tensor_tensor(agg[:, ww, :], agg[:, ww, :],
                                                pw[:], mybir.AluOpType.add)
                for g in gdone:
                    wlo = g * GW
                    whi = min(wlo + GW, cfg.t)
                    nw = whi - wlo
                    # fnext = fprev - dinv*agg
                    nc.vector.tensor_tensor(
                        fnext[:, wlo:whi, :], agg[:, wlo:whi, :],
                        dinv_s[:, wlo:whi, None].to_broadcast([P, nw, 64]),
                        mybir.AluOpType.mult)
                    nc.vector.tensor_tensor(
                        fnext[:, wlo:whi, :], fprev[:, wlo:whi, :],
                        fnext[:, wlo:whi, :], mybir.AluOpType.subtract)
                    if rnd == 0:
                        # f0 := f0 - f1 (filters); round-2 table from f1
                        nc.vector.tensor_tensor(
                            f0[:, wlo:whi, :], f0[:, wlo:whi, :],
                            f1[:, wlo:whi, :], mybir.AluOpType.subtract)
                        table_group(1, f1, g)
                    else:
                        # filters + output MLP for this group
                        j0 = wlo * P
                        w = whi * P - j0
                        zl = psum.tile([H, CH], F32, tag="A")
                        z1 = psum.tile([H, CH], F32, tag="B")
                        z2 = psum1.tile([H, CH], F32, tag="C")
                        for i in range(nw):
                            t = wlo + i
                            cs = slice(i * P, (i + 1) * P)
                            nc.tensor.matmul(zl[:, cs], f0[:, t, :], sid3[:],
                                             start=True, stop=False)
                            nc.tensor.matmul(zl[:, cs], f2[:, t, :], sid075[:],
                                             start=False, stop=True)
                            nc.tensor.matmul(z1[:, cs], f1[:, t, :], sid3[:],
                                             start=True, stop=False)
                            nc.tensor.matmul(z1[:, cs], f2[:, t, :], sidm15[:],
                                             start=False, stop=True)
                            nc.tensor.matmul(z2[:, cs], f2[:, t, :], sid075[:],
                                             start=True, stop=True)
                        zlc = io.tile([H, CH], BF16, tag="zlc")
                        zhc = io.tile([P, CH], BF16, tag="zhc")
                        nc.scalar.activation(zlc[:, :w], zl[:, :w], cp)
                        nc.scalar.activation(zhc[:H, :w], z1[:, :w], cp)
                        nc.scalar.activation(zhc[H:, :w], z2[:, :w], cp)
                        pl = psum1.tile([H, CH], F32, tag="C")
                        ph = psum.tile([H, CH], F32, tag="A")
                        nc.tensor.matmul(pl[:, :w], Ws["W3"][:], zlc[:, :w],
                                         start=True, stop=True)
                        nc.tensor.matmul(ph[:, :w], Ws["W4"][:], zhc[:, :w],
                                         start=True, stop=True)
                        ol = io.tile([H, CH], F32, tag="ol")
                        oh = io.tile([H, CH], F32, tag="oh")
                        nc.scalar.activation(ol[:, :w], pl[:, :w], relu,
                                             bias=bs["b3"][:])
                        nc.scalar.activation(oh[:, :w], ph[:, :w], relu,
                                             bias=bs["b4"][:])
                        nc.sync.dma_start(outl_d[:, j0:j0 + w], ol[:, :w])
                        nc.sync.dma_start(outh_d[:, j0:j0 + w], oh[:, :w])
                if rnd == 0 and bi == p0_fire:
                    collective(1, 0)
            if rnd == 0:
                collective(1, 1)

    nc.compile()
    return nc


# ---------------------------------------------------------------- driver

_CACHE = {}


def run(cfg, inputs, run_fn=None, **spmd_kwargs):
    in_maps, plan, total_tiles = preprocess(cfg, **inputs)
    key = (cfg.n_nodes, cfg.n_edges, cfg.nc, cfg.max_span_tiles,
           total_tiles, repr(plan))
    if key not in _CACHE:
        _CACHE[key] = build_nc(cfg, plan, total_tiles)
    nc = _CACHE[key]
    if run_fn is not None:
        results = run_fn(nc, in_maps)
        res = None
    else:
        res = run_bass_kernel_spmd(nc, in_maps, core_ids=list(range(cfg.nc)),
                                   **spmd_kwargs)
        results = res.results
    h_l = np.zeros((cfg.n_nodes, cfg.h), np.float32)
    h_h = np.zeros((cfg.n_nodes, cfg.h), np.float32)
    for c in range(cfg.nc):
        lo = c * cfg.shard
        h_l[lo:lo + cfg.shard] = results[c]["out_l"].T[:cfg.shard]
        h_h[lo:lo + cfg.shard] = results[c]["out_h"].T[:cfg.shard]
    return h_l, h_h, res


def kernel(in_feat, src, dst, W1, b1, W2, b2, W3, b3, W4, b4):
    cfg = Cfg(100000, 1600000, 128, 64, 8)
    h_l, h_h, _ = run(cfg, dict(
        in_feat=np.asarray(in_feat, np.float32),
        src=np.asarray(src, np.int64), dst=np.asarray(dst, np.int64),
        W1=np.asarray(W1, np.float32), b1=np.asarray(b1, np.float32),
        W2=np.asarray(W2, np.float32), b2=np.asarray(b2, np.float32),
        W3=np.asarray(W3, np.float32), b3=np.asarray(b3, np.float32),
        W4=np.asarray(W4, np.float32), b4=np.asarray(b4, np.float32)))
    return h_l, h_h


# revision 23
# speedup vs baseline: 1.3807x; 1.0037x over previous
"""BWGNN (Bernstein-wavelet GNN) Trainium2 kernel, 8-core SPMD.

Sharding: nodes split 8 ways (graph/data parallel); edges partitioned by dst
shard; tiny weights replicated.  Per round of Laplacian message passing the
node-state table (dinv * f, bf16, rows padded to 256B) is distributed with
AllGather, then per-edge src rows are fetched with dma_gather (int16
indices, 1024 per instruction).

Gather descriptor generation on the Pool engine (~7.9ns/idx ucode) is the
hard bottleneck, so everything else hides under it:
- edge-slot streams are padded only per chunk (max over cores, ~0.7%);
  windows float: each canonical tile carries up to 4 candidate dst windows
  (union over cores), turned into one-hot indicators on the VectorEngine
  (iota is_eq vs 4 host bf16 dstw streams, self-masking), accumulated per
  window event on the TensorEngine in PSUM, first-touch-copied into an
  SBUF fp32 agg (no memset).
- the table is split in two parts (dst windows 0-47 / 48-97+zero row) with
  separate AllGathers, so part 0's collective overlaps the previous
  round's compute; bundles are interleaved across the 4 (part, core-half)
  chunks so windows complete progressively, and per 4-window group the
  next round's table mult + DMA (and in the last round the Bernstein
  filters + output MLP + output DMA) are emitted in-round.
All matmul operands are bf16 (PSUM fp32; agg, dinv, biases, outputs fp32).
"""

import sys
from contextlib import ExitStack

import numpy as np
import ml_dtypes

try:
    import concourse  # noqa: F401
except ImportError:  # pragma: no cover
    sys.path.insert(0, "/opt/trn_rl_repo")

import concourse.bacc as bacc
import concourse.bass as bass
import concourse.mybir as mybir
import concourse.tile as tile
from concourse.bass_utils import run_bass_kernel_spmd
from concourse.library_config import mlp
from concourse.masks import make_identity

P = 128
F32 = mybir.dt.float32
BF16 = mybir.dt.bfloat16
I16 = mybir.dt.int16
BF = ml_dtypes.bfloat16
NSTREAM = 4
GW = 4                    # windows per group (= 512 cols)


class Cfg:
    def __init__(self, n_nodes, n_edges, in_feats, h_feats, n_cores,
                 max_span_tiles=8, mm_chunk=512):
        assert n_nodes % n_cores == 0
        self.n_nodes, self.n_edges = n_nodes, n_edges
        self.in_feats, self.h = in_feats, h_feats
        self.nc = n_cores
        self.shard = n_nodes // n_cores
        self.sp = ((self.shard + P - 1) // P) * P      # padded shard 12544
        self.t = self.sp // P                          # node tiles 98
        self.tp = self.sp + P                          # table rows/core 12672
        # part split: NPART window ranges; last part includes the zero tile
        self.npart = 4
        self.wpp = (self.t // self.npart // GW) * GW   # 24 windows/part
        self.pstart = [p * self.wpp for p in range(self.npart)]    # windows
        self.prow = [self.wpp * P] * (self.npart - 1)
        self.prow.append(self.tp - self.pstart[-1] * P)            # 3456
        self.zero_off = self.sp - self.pstart[-1] * P              # 3328
        # 2*npart chunks: (part, core-half)
        self.half = n_cores // 2
        self.nchunk = 2 * self.npart
        assert all(self.half * r <= 32640 for r in self.prow)
        self.n_groups = -(-self.t // GW)               # 25
        self.gpp = self.wpp // GW                      # groups per part 6
        self.max_span_tiles = max_span_tiles
        self.mm_chunk = mm_chunk
        assert mm_chunk == GW * P


# ---------------------------------------------------------------- host prep

def _wrap16(x):
    """flat int16 stream -> [128, n/16]: storage[p, col] = x[col*16 + p%16]."""
    assert len(x) % 16 == 0
    return np.tile(x.reshape(-1, 16).T, (8, 1)).copy()


def preprocess(cfg, in_feat, src, dst, W1, b1, W2, b2, W3, b3, W4, b4):
    n = cfg.n_nodes
    deg = np.bincount(dst, minlength=n).astype(np.float32)
    dinv = np.clip(deg, 1.0, None) ** -0.5

    shard_of = dst // cfg.shard
    streams = []
    for c in range(cfg.nc):
        m = shard_of == c
        s_c, d_c = src[m].astype(np.int64), dst[m].astype(np.int64)
        score = s_c // cfg.shard                       # owner core of src
        r = s_c % cfg.shard                            # local node row
        part = np.minimum((r // P) // cfg.wpp, cfg.npart - 1)
        prow = np.array(cfg.prow)[part]
        row = ((score % cfg.half) * prow
               + (r - np.array(cfg.pstart)[part] * P))
        q = part * 2 + score // cfg.half               # chunk index
        dstloc = d_c - c * cfg.shard
        o = np.lexsort((dstloc, q))
        streams.append((row[o], q[o], dstloc[o]))

    # canonical chunk lengths (max over cores, tile-rounded)
    chunk_lens = []
    for cc in range(cfg.nchunk):
        L = max(int((q == cc).sum()) for _, q, _ in streams)
        chunk_lens.append(-(-L // P) * P)
    total_tiles = sum(chunk_lens) // P
    tslots = total_tiles * P

    # per-core slot streams (slot order: chunk-major in chunk index order)
    PADV = 10 ** 9
    pad_row = [cfg.zero_off if cc // 2 == cfg.npart - 1 else 0
               for cc in range(cfg.nchunk)]            # masked anyway
    gz = np.zeros((cfg.nc, tslots), np.int32)
    dl = np.full((cfg.nc, tslots), PADV, np.int64)
    cbase = {}
    base = 0
    for cc in range(cfg.nchunk):
        cbase[cc] = base
        for c in range(cfg.nc):
            row, q, dstloc = streams[c]
            m = q == cc
            k = int(m.sum())
            gz[c, base:base + k] = row[m]
            gz[c, base + k:base + chunk_lens[cc]] = pad_row[cc]
            dl[c, base:base + k] = dstloc[m]
        base += chunk_lens[cc]

    # per-tile window sets (union over cores)
    W = dl // P
    tile_wins = []
    for t in range(total_tiles):
        wins = np.unique(W[:, t * P:(t + 1) * P])
        wins = sorted(int(w) for w in wins if w < cfg.t)
        assert len(wins) <= NSTREAM, (t, wins)
        tile_wins.append(wins)

    # per-chunk bundle lists
    cb = []
    for cc in range(cfg.nchunk):
        t0, t1 = cbase[cc] // P, (cbase[cc] + chunk_lens[cc]) // P
        tiles_of = {}
        for t in range(t0, t1):
            for j, w in enumerate(tile_wins[t]):
                tiles_of.setdefault(w, []).append((t, j))
        bl = []
        b0 = t0
        while b0 < t1:
            bt = min(cfg.max_span_tiles, t1 - b0)
            evs = []
            for w in sorted(tiles_of):
                tl = [(t - b0, j) for (t, j) in tiles_of[w]
                      if b0 <= t < b0 + bt]
                if tl:
                    evs.append((w, tl))
            bl.append((cc, bt, b0, evs))
            b0 += bt
        cb.append(bl)

    # emission order: parts 0-1 prefix, then round-robin over all chunks
    order = []
    ptr = [0] * cfg.nchunk
    PFX = 3
    for _ in range(PFX):
        for cc in (0, 1, 2, 3):
            if ptr[cc] < len(cb[cc]):
                order.append(cb[cc][ptr[cc]]); ptr[cc] += 1
    while any(ptr[cc] < len(cb[cc]) for cc in range(cfg.nchunk)):
        for cc in range(cfg.nchunk):
            if ptr[cc] < len(cb[cc]):
                order.append(cb[cc][ptr[cc]]); ptr[cc] += 1

    # first-touch + last-touch in emission order
    first_seen = set()
    last_touch = {}
    plan = []
    for bi, (cc, bt, b0, evs) in enumerate(order):
        events = []
        for (w, tl) in evs:
            ft = w not in first_seen
            first_seen.add(w)
            last_touch[w] = bi
            events.append((w, tl, ft))
        plan.append([cc, bt, b0, events, []])
    assert len(first_seen) == cfg.t, (len(first_seen), cfg.t)
    # group completion markers
    for g in range(cfg.n_groups):
        wlo, whi = g * GW, min((g + 1) * GW, cfg.t)
        done_at = max(last_touch[w] for w in range(wlo, whi))
        plan[done_at][4].append(g)

    # dstw streams: value of slot relative to its tile's j-th window
    dws = np.full((NSTREAM, cfg.nc, tslots), 999.0, np.float64)
    tl_idx = np.arange(tslots) // P
    for j in range(NSTREAM):
        wj = np.array([tile_wins[t][j] if len(tile_wins[t]) > j else -10
                       for t in range(total_tiles)])
        v = dl - (wj[tl_idx] * P)[None, :]
        dws[j] = np.where((v >= 0) & (v < P) & (dl < PADV), v, 999.0)

    in_maps = []
    for c in range(cfg.nc):
        lo, hi = c * cfg.shard, (c + 1) * cfg.shard
        xT = np.zeros((cfg.in_feats, cfg.sp), BF)
        xT[:, :cfg.shard] = in_feat[lo:hi].T.astype(BF)
        full = np.ones(cfg.sp, np.float32)
        full[:cfg.shard] = dinv[lo:hi]
        dpm = np.ascontiguousarray(full.reshape(cfg.t, P).T)
        # dstw_all [P, NSTREAM, total_tiles]
        da = np.stack([
            np.ascontiguousarray(dws[j, c].reshape(total_tiles, P).T)
            for j in range(NSTREAM)], axis=1).astype(BF)
        in_maps.append({
            "xT": xT, "dinv_pm": dpm,
            "gidx": _wrap16(gz[c].astype(np.int16)),
            "dstw": np.ascontiguousarray(da),
            "W1": np.asarray(W1, BF), "W2": np.asarray(W2, BF),
            "W3": np.asarray(W3, BF), "W4": np.asarray(W4, BF),
            "b1": np.asarray(b1, np.float32).reshape(-1, 1),
            "b2": np.asarray(b2, np.float32).reshape(-1, 1),
            "b3": np.asarray(b3, np.float32).reshape(-1, 1),
            "b4": np.asarray(b4, np.float32).reshape(-1, 1),
        })
    return in_maps, plan, total_tiles


# ---------------------------------------------------------------- builder

def build_nc(cfg, plan, total_tiles):
    H = cfg.h
    ST = cfg.max_span_tiles
    idx_cols = total_tiles * 8
    nc = bacc.Bacc("TRN2", target_bir_lowering=False, debug=False,
                   num_devices=cfg.nc)
    xT_d = nc.dram_tensor("xT", [cfg.in_feats, cfg.sp], BF16, kind="ExternalInput")
    dinv_d = nc.dram_tensor("dinv_pm", [P, cfg.t], F32, kind="ExternalInput")
    gidx_d = nc.dram_tensor("gidx", [P, idx_cols], I16, kind="ExternalInput")
    dstw_d = nc.dram_tensor("dstw", [P, NSTREAM, total_tiles], BF16,
                            kind="ExternalInput")
    W_d = {w: nc.dram_tensor(w, [cfg.in_feats if w in ("W1", "W4") else H, H],
                             BF16, kind="ExternalInput")
           for w in ("W1", "W2", "W3", "W4")}
    b_d = {b: nc.dram_tensor(b, [H, 1], F32, kind="ExternalInput")
           for b in ("b1", "b2", "b3", "b4")}
    outl_d = nc.dram_tensor("out_l", [H, cfg.sp], F32, kind="ExternalOutput")
    outh_d = nc.dram_tensor("out_h", [H, cfg.sp], F32, kind="ExternalOutput")

    relu = mybir.ActivationFunctionType.Relu
    cp = mybir.ActivationFunctionType.Copy

    with tile.TileContext(nc) as tc, ExitStack() as ctx:
        pers = ctx.enter_context(tc.tile_pool(name="pers", bufs=1))
        dram = ctx.enter_context(tc.tile_pool(name="dram", bufs=1, space="DRAM"))
        io = ctx.enter_context(tc.tile_pool(name="io", bufs=2))
        idxp = ctx.enter_context(tc.tile_pool(name="idxp", bufs=8))
        gbp = ctx.enter_context(tc.tile_pool(name="gbp", bufs=6))
        gbi = ctx.enter_context(tc.tile_pool(name="gbi", bufs=4))
        psum = ctx.enter_context(tc.tile_pool(name="psum", bufs=2, space="PSUM"))
        psum1 = ctx.enter_context(tc.tile_pool(name="psum1", bufs=2, space="PSUM"))
        psum2 = ctx.enter_context(tc.tile_pool(name="psum2", bufs=2, space="PSUM"))

        nc.gpsimd.load_library(mlp)

        f0 = pers.tile([P, cfg.t, 64], BF16, tag="f0")
        f1 = pers.tile([P, cfg.t, 64], BF16, tag="f1")
        f2 = pers.tile([P, cfg.t, 64], BF16, tag="f2")
        tbl = pers.tile([P, cfg.t + 1, 128], BF16, tag="tbl")
        dinv_s = pers.tile([P, cfg.t], F32, tag="dinv")
        dinv_bf = pers.tile([P, cfg.t], BF16, tag="dinv_bf")
        Ws = {w: pers.tile([cfg.in_feats if w in ("W1", "W4") else H, H],
                           BF16, tag=w, name=w + "_s")
              for w in ("W1", "W2", "W3", "W4")}
        bs = {b: pers.tile([H, 1], F32, tag=b, name=b + "_s")
              for b in ("b1", "b2", "b3", "b4")}
        ident = pers.tile([P, P], BF16, tag="ident")
        sid3 = pers.tile([P, P], BF16, tag="sid3")
        sid075 = pers.tile([P, P], BF16, tag="sid075")
        sidm15 = pers.tile([P, P], BF16, tag="sidm15")
        agg = pers.tile([P, cfg.t, 64], F32, tag="agg")
        iota_f = pers.tile([P, P], BF16, tag="iota_f")

        # DRAM tables: per round, npart parts
        tb_in = [[dram.tile([cfg.prow[p], 128], BF16, name=f"tb_in{r}p{p}")
                  for p in range(cfg.npart)] for r in range(2)]
        tb_full = [[dram.tile([cfg.prow[p] * cfg.nc, 128], BF16,
                              addr_space="Shared", name=f"tb_full{r}p{p}")
                    for p in range(cfg.npart)] for r in range(2)]

        for w in Ws:
            nc.sync.dma_start(Ws[w][:], W_d[w][:])
        for b in bs:
            nc.sync.dma_start(bs[b][:], b_d[b][:])
        nc.sync.dma_start(dinv_s[:], dinv_d[:])
        make_identity(nc, ident[:])
        nc.vector.tensor_scalar_mul(sid3[:], ident[:], 3.0)
        nc.vector.tensor_scalar_mul(sid075[:], ident[:], 0.75)
        nc.vector.tensor_scalar_mul(sidm15[:], ident[:], -1.5)
        nc.gpsimd.memset(tbl[:, cfg.t, :], 0.0)
        ioti = pers.tile([P, P], mybir.dt.int32, tag="ioti")
        nc.gpsimd.iota(ioti[:], pattern=[[1, P]], base=0, channel_multiplier=0)
        nc.vector.tensor_copy(iota_f[:], ioti[:])
        nc.vector.tensor_copy(dinv_bf[:], dinv_s[:])
        # zero rows of both rounds' last-part tables
        for r in range(2):
            nc.sync.dma_start(
                tb_in[r][-1][cfg.zero_off:cfg.zero_off + P, :]
                .rearrange("(t p) f -> p t f", p=P),
                tbl[:, cfg.t:cfg.t + 1, :])

        def table_group(rnd, fsrc, g):
            """tbl[:, grp] = dinv*f; DMA to tb_in[rnd]; collectives at marks."""
            wlo = g * GW
            whi = min(wlo + GW, cfg.t)
            nw = whi - wlo
            nc.vector.tensor_tensor(
                tbl[:, wlo:whi, :64], fsrc[:, wlo:whi, :],
                dinv_bf[:, wlo:whi, None].to_broadcast([P, nw, 64]),
                mybir.AluOpType.mult)
            p = min((whi - 1) // cfg.wpp, cfg.npart - 1)
            off = wlo * P - cfg.pstart[p] * P
            nc.sync.dma_start(
                tb_in[rnd][p][off:off + nw * P, :]
                .rearrange("(t p) f -> p t f", p=P),
                tbl[:, wlo:whi, :])

        def collective(rnd, p):
            nc.gpsimd.collective_compute(
                "AllGather", mybir.AluOpType.bypass,
                replica_groups=[list(range(cfg.nc))],
                ins=[tb_in[rnd][p][:]], outs=[tb_full[rnd][p][:]])

        # chunk -> (tb_full part, base row within part table)
        def chunk_region(rnd, cc):
            p = cc // 2
            half = cc % 2
            rows = cfg.prow[p]
            base = half * cfg.half * rows
            return tb_full[rnd][p], base, cfg.half * rows

        # per part: groups of that part; fire bundle index (all groups done)
        part_groups = [list(range(p * cfg.gpp, (p + 1) * cfg.gpp))
                       for p in range(cfg.npart - 1)]
        part_groups.append(list(range((cfg.npart - 1) * cfg.gpp,
                                      cfg.n_groups)))
        fire_at = {}
        seen_g = set()
        for bi, (_, _, _, _, gdone) in enumerate(plan):
            seen_g.update(gdone)
            for p in range(cfg.npart - 1):
                if p not in fire_at and all(g in seen_g
                                            for g in part_groups[p]):
                    fire_at[p] = bi
        assert len(fire_at) == cfg.npart - 1

        # ---- phase 1: MLP -> f0 node-major; table 0 produced per group
        CH = cfg.mm_chunk
        n_mlp = -(-cfg.sp // CH)
        for k in range(n_mlp):
            j0 = k * CH
            w = min(CH, cfg.sp - j0)
            xc = io.tile([cfg.in_feats, CH], BF16, tag="xc")
            nc.sync.dma_start(xc[:, :w], xT_d[:, j0:j0 + w])
            ps1 = psum.tile([H, CH], F32, tag="A")
            nc.tensor.matmul(ps1[:, :w], Ws["W1"][:], xc[:, :w],
                             start=True, stop=True)
            h1c = io.tile([H, CH], BF16, tag="h1c")
            nc.scalar.activation(h1c[:, :w], ps1[:, :w], relu, bias=bs["b1"][:])
            ps2 = psum.tile([H, CH], F32, tag="B")
            nc.tensor.matmul(ps2[:, :w], Ws["W2"][:], h1c[:, :w],
                             start=True, stop=True)
            h2c = io.tile([H, CH], BF16, tag="h2c")
            nc.scalar.activation(h2c[:, :w], ps2[:, :w], relu, bias=bs["b2"][:])
            for i in range(w // P):
                t = (j0 + i * P) // P
                ps3 = psum1.tile([P, 64], BF16, tag="C")
                nc.tensor.transpose(ps3[:], h2c[:, i * P:(i + 1) * P],
                                    ident[:H, :H])
                nc.scalar.activation(f0[:, t, :], ps3[:], cp)
            table_group(0, f0, k)
            for p in range(cfg.npart - 1):
                if k == (p + 1) * cfg.gpp - 1:
                    collective(0, p)
        collective(0, cfg.npart - 1)

        # ---- message passing rounds
        for rnd, (fprev, fnext) in enumerate([(f0, f1), (f1, f2)]):
            for bi, (cc, btiles, goff, events, gdone) in enumerate(plan):
                tbf, cbase_, clen = chunk_region(rnd, cc)
                gi = idxp.tile([P, ST * 8], I16, tag="gi")
                nc.sync.dma_start(gi[:, :btiles * 8],
                                  gidx_d[:, goff * 8:(goff + btiles) * 8])
                dv = idxp.tile([P, NSTREAM, ST], BF16, tag="dv")
                nc.scalar.dma_start(dv[:, :, :btiles],
                                    dstw_d[:, :, goff:goff + btiles])
                gb = gbp.tile([P, ST, 128], BF16, tag="gb")
                ni = btiles * P
                nc.gpsimd.dma_gather(
                    gb[:, :btiles, :], tbf[cbase_:cbase_ + clen, :],
                    gi[:, :btiles * 8], ni, ni, 128)
                for (ww, tl, ft) in events:
                    ind = gbi.tile([P, ST, P], BF16, tag="ind")
                    i0 = 0
                    while i0 < len(tl):
                        t0, j0_ = tl[i0]
                        i1 = i0 + 1
                        while (i1 < len(tl) and tl[i1][1] == j0_
                               and tl[i1][0] == tl[i1 - 1][0] + 1):
                            i1 += 1
                        ln = i1 - i0
                        nc.vector.tensor_tensor(
                            ind[:, i0:i1, :],
                            iota_f[:, None, :].to_broadcast([P, ln, P]),
                            dv[:, j0_, t0:t0 + ln, None].to_broadcast(
                                [P, ln, P]),
                            mybir.AluOpType.is_equal)
                        i0 = i1
                    pw = psum2.tile([P, 64], F32, tag="D")
                    for i, (t, j) in enumerate(tl):
                        nc.tensor.matmul(pw[:], ind[:, i, :],
                                         gb[:, t, :64],
                                         start=(i == 0), stop=(i == len(tl) - 1))
                    if ft:
                        nc.vector.tensor_copy(agg[:, ww, :], pw[:])
                    else:
                        nc.vector.tensor_tensor(agg[:, ww, :], agg[:, ww, :],
                                                pw[:], mybir.AluOpType.add)
                for g in gdone:
                    wlo = g * GW
                    whi = min(wlo + GW, cfg.t)
                    nw = whi - wlo
                    # fnext = fprev - dinv*agg
                    nc.vector.tensor_tensor(
                        fnext[:, wlo:whi, :], agg[:, wlo:whi, :],
                        dinv_s[:, wlo:whi, None].to_broadcast([P, nw, 64]),
                        mybir.AluOpType.mult)
                    nc.vector.tensor_tensor(
                        fnext[:, wlo:whi, :], fprev[:, wlo:whi, :],
                        fnext[:, wlo:whi, :], mybir.AluOpType.subtract)
                    if rnd == 0:
                        # f0 := f0 - f1 (filters); round-2 table from f1
                        nc.vector.tensor_tensor(
                            f0[:, wlo:whi, :], f0[:, wlo:whi, :],
                            f1[:, wlo:whi, :], mybir.AluOpType.subtract)
                        table_group(1, f1, g)
                    else:
                        # filters + output MLP for this group
                        j0 = wlo * P
                        w = whi * P - j0
                        zl = psum.tile([H, CH], F32, tag="A")
                        z1 = psum.tile([H, CH], F32, tag="B")
                        z2 = psum1.tile([H, CH], F32, tag="C")
                        for i in range(nw):
                            t = wlo + i
                            cs = slice(i * P, (i + 1) * P)
                            nc.tensor.matmul(zl[:, cs], f0[:, t, :], sid3[:],
                                             start=True, stop=False)
                            nc.tensor.matmul(zl[:, cs], f2[:, t, :], sid075[:],
                                             start=False, stop=True)
                            nc.tensor.matmul(z1[:, cs], f1[:, t, :], sid3[:],
                                             start=True, stop=False)
                            nc.tensor.matmul(z1[:, cs], f2[:, t, :], sidm15[:],
                                             start=False, stop=True)
                            nc.tensor.matmul(z2[:, cs], f2[:, t, :], sid075[:],
                                             start=True, stop=True)
                        zlc = io.tile([H, CH], BF16, tag="zlc")
                        zhc = io.tile([P, CH], BF16, tag="zhc")
                        nc.scalar.activation(zlc[:, :w], zl[:, :w], cp)
                        nc.scalar.activation(zhc[:H, :w], z1[:, :w], cp)
                        nc.scalar.activation(zhc[H:, :w], z2[:, :w], cp)
                        pl = psum1.tile([H, CH], F32, tag="C")
                        ph = psum.tile([H, CH], F32, tag="A")
                        nc.tensor.matmul(pl[:, :w], Ws["W3"][:], zlc[:, :w],
                                         start=True, stop=True)
                        nc.tensor.matmul(ph[:, :w], Ws["W4"][:], zhc[:, :w],
                                         start=True, stop=True)
                        ol = io.tile([H, CH], F32, tag="ol")
                        oh = io.tile([H, CH], F32, tag="oh")
                        nc.scalar.activation(ol[:, :w], pl[:, :w], relu,
                                             bias=bs["b3"][:])
                        nc.scalar.activation(oh[:, :w], ph[:, :w], relu,
                                             bias=bs["b4"][:])
                        nc.sync.dma_start(outl_d[:, j0:j0 + w], ol[:, :w])
                        nc.sync.dma_start(outh_d[:, j0:j0 + w], oh[:, :w])
                if rnd == 0:
                    for p in range(cfg.npart - 1):
                        if fire_at[p] == bi:
                            collective(1, p)
            if rnd == 0:
                collective(1, cfg.npart - 1)

    nc.compile()
    return nc


# ---------------------------------------------------------------- driver

_CACHE = {}


def run(cfg, inputs, run_fn=None, **spmd_kwargs):
    in_maps, plan, total_tiles = preprocess(cfg, **inputs)
    key = (cfg.n_nodes, cfg.n_edges, cfg.nc, cfg.max_span_tiles,
           total_tiles, repr(plan))
    if key not in _CACHE:
        _CACHE[key] = build_nc(cfg, plan, total_tiles)
    nc = _CACHE[key]
    if run_fn is not None:
        results = run_fn(nc, in_maps)
        res = None
    else:
        res = run_bass_kernel_spmd(nc, in_maps, core_ids=list(range(cfg.nc)),
                                   **spmd_kwargs)
        results = res.results
    h_l = np.zeros((cfg.n_nodes, cfg.h), np.float32)
    h_h = np.zeros((cfg.n_nodes, cfg.h), np.float32)
    for c in range(cfg.nc):
        lo = c * cfg.shard
        h_l[lo:lo + cfg.shard] = results[c]["out_l"].T[:cfg.shard]
        h_h[lo:lo + cfg.shard] = results[c]["out_h"].T[:cfg.shard]
    return h_l, h_h, res


def kernel(in_feat, src, dst, W1, b1, W2, b2, W3, b3, W4, b4):
    cfg = Cfg(100000, 1600000, 128, 64, 8)
    h_l, h_h, _ = run(cfg, dict(
        in_feat=np.asarray(in_feat, np.float32),
        src=np.asarray(src, np.int64), dst=np.asarray(dst, np.int64),
        W1=np.asarray(W1, np.float32), b1=np.asarray(b1, np.float32),
        W2=np.asarray(W2, np.float32), b2=np.asarray(b2, np.float32),
        W3=np.asarray(W3, np.float32), b3=np.asarray(b3, np.float32),
        W4=np.asarray(W4, np.float32), b4=np.asarray(b4, np.float32)))
    return h_l, h_h
